# revision 1
# baseline (speedup 1.0000x reference)
"""MLA (multi-head latent attention) Bass kernel for 8 trn2 NeuronCores.

Sharding: core = b*4 + g  (b in {0,1} batches, g in {0..3} head-groups of 4 heads).
Each core computes, for its batch b and 4 heads:
  - projections in feature-major ("transposed") layout from xT (float32r matmuls),
  - flash-style causal attention with scores computed k-major (S^T) so the
    exp'd probabilities feed the PV matmul directly (no transposes),
  - LOBO softmax: attn = exp(s) / (sum_k exp(s) + C*exp(max_k s)); the row max
    is recovered as max_k exp(s) via a DMA max-accumulate (CCE) into a per-head
    comb tile + a DVE 32x32-transpose reduction,
  - row-parallel output projection -> partial [T, E] f32.
Host sums the 4 partials per batch (the all-reduce of the row-parallel design).
"""

import math
import os

import numpy as np

import concourse.bass as bass
import concourse.mybir as mybir
import concourse.tile as _tile_mod
from concourse.tile import TileContext
from concourse.vector_clock import ScopedClock, VectorClock
import bass_rust as _bass_rust
from concourse.bass_utils import run_bass_kernel_spmd

_N_PROCS = _bass_rust.N_PROCS


def _split_drain_and_barrier(self, tick_clock, wait_clock):
    """Replacement for TileContext._drain_and_barrier: the stock version puts
    the whole global vector clock (up to 27 sem waits) on one Drain, which this
    walrus rejects ("Too many sync wait commands").  Emit one Drain per
    outstanding processor instead."""
    gc = tick_clock.global_clock
    procs = [p for p in range(_N_PROCS) if gc[p] > 0]
    for p in procs:
        vc = VectorClock([gc[q] if q == p else 0 for q in range(_N_PROCS)])
        d = self.nc.sync.drain()
        wait_clock.add_sem_waits(d.ins, ScopedClock({None: vc}))
    self.nc.all_engine_barrier()
    popped = self.nc._tile_sem_poison_stack.pop()
    assert popped is self._sem_poison
    self.nc.clear_and_free_semaphores(list(self.sems.allocated().values()))
    self.nc.all_engine_barrier()


_tile_mod.TileContext._drain_and_barrier = _split_drain_and_barrier

# ---------------------------------------------------------------------------
# This walrus build enforces small per-instruction sync-wait budgets
# ("Too many sync wait commands").  Post-process the BIR JSON: any
# instruction carrying more than its budget of waits gets the excess
# hoisted onto same-engine Drain carriers inserted immediately before it
# (same program point on the engine's sequential stream -> semantics
# unchanged).
# ---------------------------------------------------------------------------
_orig_to_json_bytes = bass.Bass.to_json_bytes
_WAIT_LIMITS = {"Drain": 1, "DMACopy": 1}
_DEF_WAIT_LIMIT = 1


def _to_json_split_waits(self, *a, **kw):
    import json as _json
    data = _json.loads(_orig_to_json_bytes(self, *a, **kw))
    nid = 0
    for f in data.get("functions", []):
        for bb in f.get("blocks", []):
            out = []
            for inst in bb.get("instructions", []):
                si = inst.get("sync_info")
                if isinstance(si, dict):
                    w = si.get("on_wait")
                    if isinstance(w, list):
                        k = _WAIT_LIMITS.get(inst.get("opcode"), _DEF_WAIT_LIMIT)
                        if len(w) > k:
                            extra, keep = w[:-k], w[-k:]
                            for wt in extra:
                                out.append({
                                    "debug": inst.get("debug"),
                                    "engine": inst["engine"],
                                    "ins": [], "outs": [],
                                    "name": f"wsplit-{nid}",
                                    "opcode": "Drain",
                                    "sync_info": {"on_update": [],
                                                  "on_wait": [wt]},
                                })
                                nid += 1
                            si["on_wait"] = keep
                out.append(inst)
            bb["instructions"] = out
    return _json.dumps(data).encode()


bass.Bass.to_json_bytes = _to_json_split_waits

B, T, E = 2, 2048, 1024
H, DH = 16, 64
DKV = 256
DR = 32
HL = 4              # heads per core
NG = 4              # head groups
SCALE = 1.0 / math.sqrt(DH + DR)
TG = 512            # query-group width
KC = 128            # key-chunk width
NTG = T // TG       # 4
NKC = T // KC       # 16
EC = E // 128       # 8  e-chunks
CC = DKV // 128     # 2  latent chunks

F32 = mybir.dt.float32
F32R = mybir.dt.float32r
BF16 = mybir.dt.bfloat16
AF = mybir.ActivationFunctionType
ALU = mybir.AluOpType
AX = mybir.AxisListType

_CACHE = {}


def _r(ap):
    return ap.bitcast(F32R)


def _build_program():
    nc = bass.Bass()

    xT = nc.declare_dram_parameter("xT", [E, T], F32, isOutput=False)
    wq = nc.declare_dram_parameter("wq", [E, HL * DH], F32, isOutput=False)
    wqr = nc.declare_dram_parameter("wqr", [E, HL * DR], F32, isOutput=False)
    wkr = nc.declare_dram_parameter("wkr", [E, DR], F32, isOutput=False)
    wkvd = nc.declare_dram_parameter("wkvd", [E, DKV], F32, isOutput=False)
    wku = nc.declare_dram_parameter("wku", [DKV, HL * DH], F32, isOutput=False)
    wvu = nc.declare_dram_parameter("wvu", [DKV, HL * DH], F32, isOutput=False)
    wo = nc.declare_dram_parameter("wo", [HL * DH, E], F32, isOutput=False)
    cosq = nc.declare_dram_parameter("cosq", [HL * DR, T], F32, isOutput=False)
    sinq = nc.declare_dram_parameter("sinq", [HL * DR, T], F32, isOutput=False)
    lobo = nc.declare_dram_parameter("lobo", [HL, 1], F32, isOutput=False)
    masks = nc.declare_dram_parameter("masks", [128, 4 * TG], F32, isOutput=False)
    out = nc.declare_dram_parameter("out", [T, E], F32, isOutput=True)

    with TileContext(nc) as tc:
        from contextlib import ExitStack

        with ExitStack() as ctx:
            singles = ctx.enter_context(tc.tile_pool(name="singles", bufs=1))
            pool = ctx.enter_context(tc.tile_pool(name="pool", bufs=2))
            psp = ctx.enter_context(tc.tile_pool(name="psp", bufs=1, space="PSUM"))

            # ---------------- weights (f32; x-side used as f32r) ----------------
            wq_sb = singles.tile([128, EC, HL * DH], BF16)
            nc.gpsimd.dma_start(
                out=wq_sb, in_=wq.rearrange("(c p) f -> p c f", p=128))
            wqr_sb = singles.tile([128, EC, HL * DR], BF16)
            nc.gpsimd.dma_start(
                out=wqr_sb, in_=wqr.rearrange("(c p) f -> p c f", p=128))
            wkr_sb = singles.tile([128, EC, DR], BF16)
            nc.gpsimd.dma_start(
                out=wkr_sb, in_=wkr.rearrange("(c p) f -> p c f", p=128))
            wkvd_sb = singles.tile([128, EC, DKV], BF16)
            nc.gpsimd.dma_start(
                out=wkvd_sb, in_=wkvd.rearrange("(c p) f -> p c f", p=128))
            # latent-side weights in bf16 (latT is bf16)
            wku_sb = singles.tile([128, CC, HL * DH], BF16)
            nc.gpsimd.dma_start(
                out=wku_sb, in_=wku.rearrange("(c p) f -> p c f", p=128))
            wvu_sb = singles.tile([128, CC, HL * DH], BF16)
            nc.gpsimd.dma_start(
                out=wvu_sb, in_=wvu.rearrange("(c p) f -> p c f", p=128))
            wo_sb = singles.tile([128, 2, E], BF16)
            nc.gpsimd.dma_start(
                out=wo_sb, in_=wo.rearrange("(c p) e -> p c e", p=128))

            cosq_sb = singles.tile([128, T], BF16)
            nc.gpsimd.dma_start(out=cosq_sb, in_=cosq[:, :])
            sinq_sb = singles.tile([128, T], BF16)
            nc.gpsimd.dma_start(out=sinq_sb, in_=sinq[:, :])
            lobo_sb = singles.tile([HL, 1], F32)
            nc.sync.dma_start(out=lobo_sb, in_=lobo[:, :])
            c_sb = singles.tile([HL, 1], F32)
            nc.scalar.activation(c_sb, lobo_sb, AF.Exp)

            # causal masks for the 4 diagonal offsets: keep iff x - y - 128*j <= 0
            masks_sb = singles.tile([128, 4, TG], BF16)
            nc.gpsimd.dma_start(
                out=masks_sb, in_=masks.rearrange("p (j y) -> p j y", j=4))

            ones_sb = singles.tile([1, DH], F32)
            nc.vector.memset(ones_sb, 1.0)

            # ---------------- persistent activation tiles ----------------
            latT_sb = singles.tile([128, CC, T], BF16)
            qT = [singles.tile([96, T], BF16, name=f"qT{h}") for h in range(HL)]
            kT = [singles.tile([96, T], BF16, name=f"kT{h}") for h in range(HL)]
            rp_pre = singles.tile([128, T], BF16)
            rp_swap = singles.tile([128, T], BF16)
            rp_m1 = singles.tile([128, T], BF16)
            rp_m2 = singles.tile([128, T], BF16)
            kr_pre = singles.tile([DR, T], BF16)
            xt_sb = singles.tile([128, EC, T], BF16)
            nc.gpsimd.dma_start(
                out=xt_sb, in_=xT.rearrange("(c p) t -> p c t", p=128))

            # ---------------- projections from xT, streamed per tg ----------------
            for tg in range(NTG):
                ts = slice(tg * TG, (tg + 1) * TG)
                xts = [xt_sb[:, ec, ts] for ec in range(EC)]
                # latent halves + k_rope
                pa = psp.tile([128, TG], F32, name="pa", tag="A", bufs=3)
                pb = psp.tile([128, TG], F32, name="pb", tag="B", bufs=3)
                pc = psp.tile([128, TG], F32, name="pc", tag="C", bufs=2)
                for ec in range(EC):
                    nc.tensor.matmul(
                        pa, (wkvd_sb[:, ec, 0:128]), (xts[ec]),
                        start=(ec == 0), stop=(ec == EC - 1))
                    nc.tensor.matmul(
                        pb, (wkvd_sb[:, ec, 128:256]), (xts[ec]),
                        start=(ec == 0), stop=(ec == EC - 1))
                    nc.tensor.matmul(
                        pc[0:DR, :], (wkr_sb[:, ec, :]), (xts[ec]),
                        start=(ec == 0), stop=(ec == EC - 1))
                nc.vector.tensor_copy(latT_sb[:, 0, ts], pa)
                nc.vector.tensor_copy(latT_sb[:, 1, ts], pb)
                nc.scalar.copy(kr_pre[:, ts], pc[0:DR, :])
                # q projections
                pa = psp.tile([128, TG], F32, name="pa", tag="A", bufs=3)
                pb = psp.tile([128, TG], F32, name="pb", tag="B", bufs=3)
                pc = psp.tile([128, TG], F32, name="pc", tag="C", bufs=2)
                for ec in range(EC):
                    nc.tensor.matmul(
                        pa, (wq_sb[:, ec, 0:128]), (xts[ec]),
                        start=(ec == 0), stop=(ec == EC - 1))
                    nc.tensor.matmul(
                        pb, (wq_sb[:, ec, 128:256]), (xts[ec]),
                        start=(ec == 0), stop=(ec == EC - 1))
                    nc.tensor.matmul(
                        pc, (wqr_sb[:, ec, :]), (xts[ec]),
                        start=(ec == 0), stop=(ec == EC - 1))
                st = pool.tile([128, TG], BF16, name="st0", tag="qkstage", bufs=3)
                nc.scalar.copy(st, pa)
                nc.sync.dma_start(out=qT[0][0:DH, ts], in_=st[0:DH, :])
                nc.sync.dma_start(out=qT[1][0:DH, ts], in_=st[DH:128, :])
                st = pool.tile([128, TG], BF16, name="st1", tag="qkstage", bufs=3)
                nc.scalar.copy(st, pb)
                nc.sync.dma_start(out=qT[2][0:DH, ts], in_=st[0:DH, :])
                nc.sync.dma_start(out=qT[3][0:DH, ts], in_=st[DH:128, :])
                nc.scalar.copy(rp_pre[:, ts], pc)
                # rope on q_r rows for this tg
                for h in range(HL):
                    nc.sync.dma_start(
                        out=rp_swap[h * DR:h * DR + 16, ts],
                        in_=rp_pre[h * DR + 16:h * DR + 32, ts])
                    nc.sync.dma_start(
                        out=rp_swap[h * DR + 16:h * DR + 32, ts],
                        in_=rp_pre[h * DR:h * DR + 16, ts])
                nc.vector.tensor_mul(rp_m1[:, ts], rp_pre[:, ts], cosq_sb[:, ts])
                nc.vector.tensor_mul(rp_m2[:, ts], rp_swap[:, ts], sinq_sb[:, ts])
                nc.vector.tensor_add(rp_m2[:, ts], rp_m1[:, ts], rp_m2[:, ts])
                for h in range(HL):
                    nc.sync.dma_start(
                        out=qT[h][DH:96, ts], in_=rp_m2[h * DR:(h + 1) * DR, ts])
                # rope on k_r rows for this tg
                nc.sync.dma_start(
                    out=rp_swap[0:16, ts], in_=kr_pre[16:32, ts])
                nc.sync.dma_start(
                    out=rp_swap[16:32, ts], in_=kr_pre[0:16, ts])
                nc.vector.tensor_mul(
                    rp_m1[0:DR, ts], kr_pre[:, ts], cosq_sb[0:DR, ts])
                nc.vector.tensor_mul(
                    rp_m2[0:DR, ts], rp_swap[0:DR, ts], sinq_sb[0:DR, ts])
                nc.vector.tensor_add(
                    rp_m2[0:DR, ts], rp_m1[0:DR, ts], rp_m2[0:DR, ts])
                for h in range(HL):
                    nc.sync.dma_start(out=kT[h][DH:96, ts], in_=rp_m2[0:DR, ts])

            # ---------------- k_c from latentT ----------------
            for hp in range(2):
                for tg in range(NTG):
                    ts = slice(tg * TG, (tg + 1) * TG)
                    pa = psp.tile([128, TG], F32, name="pa", tag="A", bufs=3)
                    for cc in range(CC):
                        nc.tensor.matmul(
                            pa, wku_sb[:, cc, hp * 128:(hp + 1) * 128],
                            latT_sb[:, cc, ts],
                            start=(cc == 0), stop=(cc == CC - 1))
                    st = pool.tile([128, TG], BF16, name="st2", tag="qkstage", bufs=3)
                    nc.vector.tensor_copy(st, pa)
                    nc.sync.dma_start(out=kT[2 * hp][0:DH, ts], in_=st[0:DH, :])
                    nc.sync.dma_start(
                        out=kT[2 * hp + 1][0:DH, ts], in_=st[DH:128, :])

            # ---------------- V (natural layout, +ones column) ----------------
            v_sb = singles.tile([128, NKC, HL, DH + 1], BF16)
            nc.vector.memset(v_sb, 1.0)
            for tt in range(NKC):
                pb = psp.tile([128, HL * DH], F32, name="pv", tag="B", bufs=3)
                for cc in range(CC):
                    nc.tensor.matmul(
                        pb, latT_sb[:, cc, tt * 128:(tt + 1) * 128],
                        wvu_sb[:, cc, :],
                        start=(cc == 0), stop=(cc == CC - 1))
                nc.vector.tensor_copy(v_sb[:, tt, :, 0:DH], pb)

            # ---------------- attention ----------------
            yraw_sb = singles.tile([DH, HL, T], BF16)
            dsum_sb = singles.tile([HL, T], F32)
            emax_sb = singles.tile([HL, T], F32)
            emst_sb = singles.tile([HL, T], F32)

            for h in range(HL):
                comb = pool.tile([128, T], BF16, name="comb", tag="comb", bufs=1)
                nc.vector.memset(comb, 0.0)
                for qg in range(NTG):
                    qs = slice(qg * TG, (qg + 1) * TG)
                    nkc = 4 * qg + 4
                    yps = psp.tile([DH + 1, TG], F32, name="py", tag="B", bufs=3)
                    for kc in range(nkc):
                        sps = psp.tile([128, TG], F32, name="ps", tag="A", bufs=3)
                        nc.tensor.matmul(
                            sps, kT[h][:, kc * KC:(kc + 1) * KC], qT[h][:, qs])
                        pt = pool.tile(
                            [128, TG], BF16, name="pt", tag="ptile", bufs=4)
                        nc.scalar.activation(pt, sps, AF.Exp, scale=SCALE)
                        j = kc - 4 * qg
                        if j >= 0:
                            nc.gpsimd.tensor_mul(pt, pt, masks_sb[:, j, :])
                        nc.vector.tensor_max(comb[:, qs], comb[:, qs], pt)
                        nc.tensor.matmul(
                            yps, v_sb[:, kc, h, :], pt,
                            start=(kc == 0), stop=(kc == nkc - 1))
                    nc.scalar.copy(yraw_sb[:, h, qs], yps[0:DH, :])
                    std = pool.tile([DH + 1, TG], F32, name="std", tag="stgd", bufs=1)
                    nc.scalar.copy(std[DH:DH + 1, :], yps[DH:DH + 1, :])
                    nc.sync.dma_start(
                        out=dsum_sb[h:h + 1, qs], in_=std[DH:DH + 1, :])
                # emax for this head: partition-max of comb via 32x32 transpose
                combT = pool.tile([128, T], BF16, name="combT", tag="combT", bufs=1)
                nc.vector.transpose(combT, comb)
                red = pool.tile([128, T // 32], F32, name="red", tag="red", bufs=1)
                nc.vector.reduce_max(
                    red, combT.rearrange("p (b j) -> p b j", j=32), axis=AX.X)
                stk = pool.tile([32, 4, T // 32], F32, name="stk", tag="stk", bufs=1)
                for a in range(4):
                    nc.sync.dma_start(
                        out=stk[:, a, :], in_=red[a * 32:(a + 1) * 32, :])
                emf = pool.tile([32, T // 32], F32, name="emf", tag="emf", bufs=1)
                nc.vector.reduce_max(
                    emf, stk.rearrange("p a b -> p b a"), axis=AX.X)
                nc.sync.dma_start(out=emst_sb[h:h + 1, :], in_=emf)

            # ---------------- denominators + normalize ----------------
            # un-permute the per-head maxes (i-major -> natural q order)
            nc.vector.tensor_copy(
                emax_sb.rearrange("p (b i) -> p i b", i=32),
                emst_sb.rearrange("p (i b) -> p i b", b=64))
            # d = dsum + C * emax  (in place into dsum), r = 1/d (into emax)
            nc.vector.scalar_tensor_tensor(
                out=dsum_sb, in0=emax_sb, scalar=c_sb, in1=dsum_sb,
                op0=ALU.mult, op1=ALU.add)
            nc.vector.reciprocal(emax_sb, dsum_sb)

            yT_sb = singles.tile([128, 2, T], BF16)
            for h in range(HL):
                for qg in range(NTG):
                    qs = slice(qg * TG, (qg + 1) * TG)
                    rhh = pool.tile([1, TG], F32, name="rh", tag="rh", bufs=2)
                    nc.sync.dma_start(out=rhh, in_=emax_sb[h:h + 1, qs])
                    bc = psp.tile([DH, TG], F32, name="bc", tag="C", bufs=2)
                    nc.tensor.matmul(bc, ones_sb, rhh)
                    yn = pool.tile([DH, TG], BF16, name="yn", tag="yn", bufs=3)
                    nc.vector.tensor_mul(yn, yraw_sb[:, h, qs], bc)
                    nc.sync.dma_start(
                        out=yT_sb[(h % 2) * DH:(h % 2 + 1) * DH, h // 2, qs],
                        in_=yn)

            # ---------------- output projection (row-parallel partial) ----------------
            for tt in range(NKC):
                for eg in range(2):
                    pa = psp.tile([128, TG], F32, name="po", tag="A", bufs=3)
                    for fc in range(2):
                        nc.tensor.matmul(
                            pa, yT_sb[:, fc, tt * 128:(tt + 1) * 128],
                            wo_sb[:, fc, eg * TG:(eg + 1) * TG],
                            start=(fc == 0), stop=(fc == 1))
                    ost = pool.tile([128, TG], F32, name="ost", tag="ost", bufs=2)
                    if (tt + eg) % 2 == 0:
                        nc.scalar.copy(ost, pa)
                    else:
                        nc.vector.tensor_copy(ost, pa)
                    nc.sync.dma_start(
                        out=out[tt * 128:(tt + 1) * 128, eg * TG:(eg + 1) * TG],
                        in_=ost)

    return nc


def _masks():
    x = np.arange(128)[:, None]
    y = np.arange(TG)[None, :]
    ms = [(x - y + 128 * j <= 0).astype(np.float32) for j in range(4)]
    return np.concatenate(ms, axis=1)  # [128, 4*TG]


def _rope_tables():
    half = DR // 2
    inv = 1.0 / (10000.0 ** (np.arange(half, dtype=np.float64) / half))
    ang = np.arange(T, dtype=np.float64)[:, None] * inv[None, :]  # (T, half)
    cos = np.cos(ang).T  # (half, T)
    sin = np.sin(ang).T
    cosk = np.concatenate([cos, cos], axis=0)                 # (32, T)
    sink = np.concatenate([-sin, sin], axis=0)
    cosq = np.tile(cosk, (HL, 1)).astype(np.float32)          # (128, T)
    sinq = np.tile(sink, (HL, 1)).astype(np.float32)
    return cosq, sinq


def kernel(x, Wq, Wqr, Wkr, Wkvd, Wku, Wvu, Wo, lobo_log):
    x = np.asarray(x, dtype=np.float32)
    Wq = np.asarray(Wq, dtype=np.float32)
    Wqr = np.asarray(Wqr, dtype=np.float32)
    Wkr = np.asarray(Wkr, dtype=np.float32)
    Wkvd = np.asarray(Wkvd, dtype=np.float32)
    Wku = np.asarray(Wku, dtype=np.float32)
    Wvu = np.asarray(Wvu, dtype=np.float32)
    Wo = np.asarray(Wo, dtype=np.float32)
    lobo_log = np.asarray(lobo_log, dtype=np.float32)

    if "nc" not in _CACHE:
        _CACHE["nc"] = _build_program()
    nc = _CACHE["nc"]

    cosq, sinq = _rope_tables()
    msk = _masks()
    in_maps = []
    for core in range(8):
        b, g = core // NG, core % NG
        hs = slice(g * HL * DH, (g + 1) * HL * DH)
        rs = slice(g * HL * DR, (g + 1) * HL * DR)
        in_maps.append({
            "xT": np.ascontiguousarray(x[b].T),
            "wq": np.ascontiguousarray(Wq[:, hs]),
            "wqr": np.ascontiguousarray(Wqr[:, rs]),
            "wkr": Wkr,
            "wkvd": Wkvd,
            "wku": np.ascontiguousarray(Wku[:, hs]),
            "wvu": np.ascontiguousarray(Wvu[:, hs]),
            "wo": np.ascontiguousarray(Wo[hs, :]),
            "cosq": cosq, "sinq": sinq, "masks": msk,
            "lobo": np.ascontiguousarray(
                lobo_log[g * HL:(g + 1) * HL].reshape(HL, 1)),
        })

    trace = bool(os.environ.get("BASS_TRACE_KERNEL"))
    bkr = run_bass_kernel_spmd(
        nc, in_maps, core_ids=list(range(8)), trace=trace)
    if trace:
        print(f"HW exec time: {bkr.exec_time_ns} ns")
        if bkr.instructions_and_trace is not None:
            print("trace:", bkr.instructions_and_trace[1])
        _CACHE["last_result"] = bkr
    res = bkr.results
    out = np.zeros((B, T, E), dtype=np.float32)
    for core in range(8):
        out[core // NG] += res[core]["out"]
    return out



# revision 9
# speedup vs baseline: 1.3852x; 1.3852x over previous
"""MLA (multi-head latent attention) Bass kernel for 8 trn2 NeuronCores.

Sharding: core = b*4 + g  (b in {0,1} batches, g in {0..3} head-groups of 4 heads).
Each core: projections from xT (bf16 matmuls), flash-style causal attention with
k-major scores (S^T) so exp'd probs feed PV directly, LOBO softmax
attn = exp(s) / (sum_k exp(s) + C*exp(max_k s)), row-parallel out-proj partial.
Host sums the 4 partials per batch.

v2 layout notes:
  - PSUM tags: S = [128,1536] (3 banks) x2, Y = [128,512] x2  -> 8 banks total.
  - Projections pack (kv0,kv1,kr) and (q01,q23,qr) each into one S tile.
  - Attention: per (h,qg) score chunks land 3-per-S-tile; one wide exp per
    off-diagonal run, granular exps + DVE-memset + gpsimd triangle-mask on the
    4 diagonal chunks.  Per-query running max kept in a [128,T] bf16 comb tile
    (DVE tensor_max); partition-max via ONE gpsimd tensor_reduce(axis=C).
  - Denominator D rides a ones-column in V through the PV matmul (row 64).
  - y normalization: r broadcast to 64-row blocks via a tiny f32r matmul with a
    0/1 selector lhsT; yT2 multiplied in place.
"""

import math
import os

import numpy as np

import concourse.bass as bass
import concourse.bass_isa as bass_isa
import concourse.mybir as mybir
import concourse.tile as _tile_mod
from concourse.tile import TileContext
from concourse.vector_clock import ScopedClock, VectorClock
import bass_rust as _bass_rust
from concourse.bass_utils import run_bass_kernel_spmd

_N_PROCS = _bass_rust.N_PROCS


def _split_drain_and_barrier(self, tick_clock, wait_clock):
    """Replacement for TileContext._drain_and_barrier: the stock version puts
    the whole global vector clock (up to 27 sem waits) on one Drain, which this
    walrus rejects ("Too many sync wait commands").  Emit one Drain per
    outstanding processor instead."""
    gc = tick_clock.global_clock
    procs = [p for p in range(_N_PROCS) if gc[p] > 0]
    for p in procs:
        vc = VectorClock([gc[q] if q == p else 0 for q in range(_N_PROCS)])
        d = self.nc.sync.drain()
        wait_clock.add_sem_waits(d.ins, ScopedClock({None: vc}))
    self.nc.all_engine_barrier()
    popped = self.nc._tile_sem_poison_stack.pop()
    assert popped is self._sem_poison
    self.nc.clear_and_free_semaphores(list(self.sems.allocated().values()))
    self.nc.all_engine_barrier()


_tile_mod.TileContext._drain_and_barrier = _split_drain_and_barrier

# ---------------------------------------------------------------------------
# This walrus build enforces small per-instruction sync-wait budgets
# ("Too many sync wait commands").  Post-process the BIR JSON: any
# instruction carrying more than its budget of waits gets the excess
# hoisted onto same-engine Drain carriers inserted immediately before it
# (same program point on the engine's sequential stream -> semantics
# unchanged).
# ---------------------------------------------------------------------------
_orig_to_json_bytes = bass.Bass.to_json_bytes
_WAIT_LIMITS = {"Drain": 1, "DMACopy": 1}
_DEF_WAIT_LIMIT = 1


def _to_json_split_waits(self, *a, **kw):
    import json as _json
    data = _json.loads(_orig_to_json_bytes(self, *a, **kw))
    nid = 0
    for f in data.get("functions", []):
        for bb in f.get("blocks", []):
            out = []
            for inst in bb.get("instructions", []):
                si = inst.get("sync_info")
                if isinstance(si, dict):
                    w = si.get("on_wait")
                    if isinstance(w, list):
                        k = _WAIT_LIMITS.get(inst.get("opcode"), _DEF_WAIT_LIMIT)
                        if len(w) > k:
                            extra, keep = w[:-k], w[-k:]
                            for wt in extra:
                                out.append({
                                    "debug": inst.get("debug"),
                                    "engine": inst["engine"],
                                    "ins": [], "outs": [],
                                    "name": f"wsplit-{nid}",
                                    "opcode": "Drain",
                                    "sync_info": {"on_update": [],
                                                  "on_wait": [wt]},
                                })
                                nid += 1
                            si["on_wait"] = keep
                out.append(inst)
            bb["instructions"] = out
    return _json.dumps(data).encode()


bass.Bass.to_json_bytes = _to_json_split_waits

B, T, E = 2, 2048, 1024
H, DH = 16, 64
DKV = 256
DR = 32
HL = 4              # heads per core
NG = 4              # head groups
SCALE = 1.0 / math.sqrt(DH + DR)
TG = 512            # query-group width
KC = 128            # key-chunk width
NTG = T // TG       # 4
NKC = T // KC       # 16
EC = E // 128       # 8  e-chunks
CC = DKV // 128     # 2  latent chunks

F32 = mybir.dt.float32
F32R = mybir.dt.float32r
BF16 = mybir.dt.bfloat16
AF = mybir.ActivationFunctionType
ALU = mybir.AluOpType
AX = mybir.AxisListType

_CACHE = {}


def _r(ap):
    return ap.bitcast(F32R)


def _build_program():
    nc = bass.Bass()

    xT = nc.declare_dram_parameter("xT", [E, T], F32, isOutput=False)
    wq = nc.declare_dram_parameter("wq", [E, HL * DH], F32, isOutput=False)
    wqr = nc.declare_dram_parameter("wqr", [E, HL * DR], F32, isOutput=False)
    wkr = nc.declare_dram_parameter("wkr", [E, DR], F32, isOutput=False)
    wkvd = nc.declare_dram_parameter("wkvd", [E, DKV], F32, isOutput=False)
    wku = nc.declare_dram_parameter("wku", [DKV, HL * DH], F32, isOutput=False)
    wvu = nc.declare_dram_parameter("wvu", [DKV, HL * DH], F32, isOutput=False)
    wo = nc.declare_dram_parameter("wo", [HL * DH, E], F32, isOutput=False)
    cosq = nc.declare_dram_parameter("cosq", [HL * DR, T], F32, isOutput=False)
    sinq = nc.declare_dram_parameter("sinq", [HL * DR, T], F32, isOutput=False)
    tri = nc.declare_dram_parameter("tri", [128, 128], F32, isOutput=False)
    sel = nc.declare_dram_parameter("sel", [66, 128], F32, isOutput=False)
    lobo = nc.declare_dram_parameter("lobo", [66, 1], F32, isOutput=False)
    out = nc.declare_dram_parameter("out", [T, E], F32, isOutput=True)

    with TileContext(nc) as tc:
        from contextlib import ExitStack

        with ExitStack() as ctx:
            singles = ctx.enter_context(tc.tile_pool(name="singles", bufs=1))
            pool = ctx.enter_context(tc.tile_pool(name="pool", bufs=2))
            psp = ctx.enter_context(tc.tile_pool(name="psp", bufs=1, space="PSUM"))

            # ---------------- weights (bf16 in SBUF) ----------------
            wq_sb = singles.tile([128, EC, HL * DH], BF16)
            nc.gpsimd.dma_start(
                out=wq_sb, in_=wq.rearrange("(c p) f -> p c f", p=128))
            wqr_sb = singles.tile([128, EC, HL * DR], BF16)
            nc.gpsimd.dma_start(
                out=wqr_sb, in_=wqr.rearrange("(c p) f -> p c f", p=128))
            wkr_sb = singles.tile([128, EC, DR], BF16)
            nc.gpsimd.dma_start(
                out=wkr_sb, in_=wkr.rearrange("(c p) f -> p c f", p=128))
            wkvd_sb = singles.tile([128, EC, DKV], BF16)
            nc.gpsimd.dma_start(
                out=wkvd_sb, in_=wkvd.rearrange("(c p) f -> p c f", p=128))
            wku_sb = singles.tile([128, CC, HL * DH], BF16)
            nc.gpsimd.dma_start(
                out=wku_sb, in_=wku.rearrange("(c p) f -> p c f", p=128))
            wvu_sb = singles.tile([128, CC, HL * DH], BF16)
            nc.gpsimd.dma_start(
                out=wvu_sb, in_=wvu.rearrange("(c p) f -> p c f", p=128))
            wo_sb = singles.tile([128, 2, E], BF16)
            nc.gpsimd.dma_start(
                out=wo_sb, in_=wo.rearrange("(c p) e -> p c e", p=128))

            cosq_sb = singles.tile([128, T], BF16)
            nc.gpsimd.dma_start(out=cosq_sb, in_=cosq[:, :])
            sinq_sb = singles.tile([128, T], BF16)
            nc.gpsimd.dma_start(out=sinq_sb, in_=sinq[:, :])
            tri_sb = singles.tile([128, 128], BF16)
            nc.gpsimd.dma_start(out=tri_sb, in_=tri[:, :])
            sel_sb = singles.tile([66, 128], BF16)
            nc.gpsimd.dma_start(out=sel_sb, in_=sel[:, :])
            lobo_sb = singles.tile([66, 1], F32)
            nc.sync.dma_start(out=lobo_sb, in_=lobo[:, :])
            c_sb = singles.tile([66, 1], F32)
            nc.scalar.activation(c_sb, lobo_sb, AF.Exp)

            # ---------------- persistent activation tiles ----------------
            xt_sb = singles.tile([128, EC, T], BF16)
            xT_r = xT.rearrange("(c p) t -> p c t", p=128)
            latT_sb = singles.tile([128, CC, T], BF16)
            qT = [singles.tile([96, T], BF16, name=f"qT{h}") for h in range(HL)]
            kT = [singles.tile([96, T], BF16, name=f"kT{h}") for h in range(HL)]
            v_sb = singles.tile([128, NKC, HL, DH + 1], BF16)
            nc.vector.memset(v_sb[:, :, :, DH:DH + 1], 1.0)
            yT2 = singles.tile([128, 2, T], BF16)
            comb = singles.tile([128, T], BF16)
            # head h row lives at partition 64*(h//2) + h%2
            dsum_sb = singles.tile([66, T], F32)
            emax_sb = singles.tile([66, T], F32)
            emst_sb = singles.tile([66, T], F32)
            nc.vector.memset(dsum_sb, 1.0)
            nc.vector.memset(emax_sb, 1.0)
            nc.vector.memset(emst_sb, 1.0)

            # =================== projections, per tg ===================
            for tg in range(NTG):
                ts = slice(tg * TG, (tg + 1) * TG)
                nc.gpsimd.dma_start(
                    out=xt_sb[:, :, ts], in_=xT_r[:, :, ts])
                xts = [xt_sb[:, ec, ts] for ec in range(EC)]

                # --- latent (kv) halves + k_rope into one S tile ---
                skv = psp.tile([128, 3 * TG], F32, name="skv", tag="S", bufs=2)
                for ec in range(EC):
                    nc.tensor.matmul(
                        skv[:, 0:TG], wkvd_sb[:, ec, 0:128], xts[ec],
                        start=(ec == 0), stop=(ec == EC - 1))
                for ec in range(EC):
                    nc.tensor.matmul(
                        skv[:, TG:2 * TG], wkvd_sb[:, ec, 128:256], xts[ec],
                        start=(ec == 0), stop=(ec == EC - 1))
                for ec in range(EC):
                    nc.tensor.matmul(
                        skv[0:DR, 2 * TG:3 * TG], wkr_sb[:, ec, :], xts[ec],
                        start=(ec == 0), stop=(ec == EC - 1))
                nc.vector.tensor_copy(
                    latT_sb[:, :, ts],
                    skv[:, 0:2 * TG].rearrange("p (c t) -> p c t", c=2))
                kr_pre = pool.tile([DR, TG], BF16, name="krp", tag="krp", bufs=2)
                nc.scalar.copy(kr_pre, skv[0:DR, 2 * TG:3 * TG])

                # k_rope rotate-half + tables
                kr_sw = pool.tile([DR, TG], BF16, name="krs", tag="krs", bufs=2)
                nc.sync.dma_start(out=kr_sw[0:16, :], in_=kr_pre[16:32, :])
                nc.sync.dma_start(out=kr_sw[16:32, :], in_=kr_pre[0:16, :])
                kr_m = pool.tile([DR, TG], BF16, name="krm", tag="krm", bufs=2)
                nc.vector.tensor_mul(kr_m, kr_pre, cosq_sb[0:DR, ts])
                nc.vector.tensor_mul(kr_sw, kr_sw, sinq_sb[0:DR, ts])
                nc.vector.tensor_add(kr_m, kr_m, kr_sw)
                for h in range(HL):
                    nc.sync.dma_start(out=kT[h][DH:96, ts], in_=kr_m)

                # --- q halves + q_rope into one S tile ---
                sq = psp.tile([128, 3 * TG], F32, name="sq", tag="S", bufs=2)
                for ec in range(EC):
                    nc.tensor.matmul(
                        sq[:, 0:TG], wq_sb[:, ec, 0:128], xts[ec],
                        start=(ec == 0), stop=(ec == EC - 1))
                for ec in range(EC):
                    nc.tensor.matmul(
                        sq[:, TG:2 * TG], wq_sb[:, ec, 128:256], xts[ec],
                        start=(ec == 0), stop=(ec == EC - 1))
                for ec in range(EC):
                    nc.tensor.matmul(
                        sq[:, 2 * TG:3 * TG], wqr_sb[:, ec, :], xts[ec],
                        start=(ec == 0), stop=(ec == EC - 1))
                stq = pool.tile([128, 2 * TG], BF16, name="stq", tag="stq", bufs=2)
                nc.scalar.copy(stq, sq[:, 0:2 * TG])
                nc.sync.dma_start(out=qT[0][0:DH, ts], in_=stq[0:DH, 0:TG])
                nc.sync.dma_start(out=qT[1][0:DH, ts], in_=stq[DH:128, 0:TG])
                nc.sync.dma_start(out=qT[2][0:DH, ts], in_=stq[0:DH, TG:2 * TG])
                nc.sync.dma_start(out=qT[3][0:DH, ts], in_=stq[DH:128, TG:2 * TG])
                rp_pre = pool.tile([128, TG], BF16, name="rpp", tag="rpp", bufs=2)
                nc.scalar.copy(rp_pre, sq[:, 2 * TG:3 * TG])
                rp_sw = pool.tile([128, TG], BF16, name="rps", tag="rps", bufs=2)
                for h in range(HL):
                    nc.sync.dma_start(
                        out=rp_sw[h * DR:h * DR + 16, :],
                        in_=rp_pre[h * DR + 16:h * DR + 32, :])
                    nc.sync.dma_start(
                        out=rp_sw[h * DR + 16:h * DR + 32, :],
                        in_=rp_pre[h * DR:h * DR + 16, :])
                rp_m = pool.tile([128, TG], BF16, name="rpm", tag="rpm", bufs=2)
                nc.vector.tensor_mul(rp_m, rp_pre, cosq_sb[:, ts])
                nc.vector.tensor_mul(rp_sw, rp_sw, sinq_sb[:, ts])
                nc.vector.tensor_add(rp_m, rp_m, rp_sw)
                for h in range(HL):
                    nc.sync.dma_start(
                        out=qT[h][DH:96, ts], in_=rp_m[h * DR:(h + 1) * DR, :])

                # --- k_c from latent ---
                skc = psp.tile([128, 3 * TG], F32, name="skc", tag="S", bufs=2)
                for cc in range(CC):
                    nc.tensor.matmul(
                        skc[:, 0:TG], wku_sb[:, cc, 0:128], latT_sb[:, cc, ts],
                        start=(cc == 0), stop=(cc == CC - 1))
                for cc in range(CC):
                    nc.tensor.matmul(
                        skc[:, TG:2 * TG], wku_sb[:, cc, 128:256],
                        latT_sb[:, cc, ts],
                        start=(cc == 0), stop=(cc == CC - 1))
                stk = pool.tile([128, 2 * TG], BF16, name="stk", tag="stq", bufs=2)
                nc.vector.tensor_copy(stk, skc[:, 0:2 * TG])
                nc.sync.dma_start(out=kT[0][0:DH, ts], in_=stk[0:DH, 0:TG])
                nc.sync.dma_start(out=kT[1][0:DH, ts], in_=stk[DH:128, 0:TG])
                nc.sync.dma_start(out=kT[2][0:DH, ts], in_=stk[0:DH, TG:2 * TG])
                nc.sync.dma_start(out=kT[3][0:DH, ts], in_=stk[DH:128, TG:2 * TG])

                # --- V (natural layout) for this tg's 4 key chunks ---
                for half in range(2):
                    kc0 = 4 * tg + 2 * half
                    vps = psp.tile([128, TG], F32, name="vps", tag="Y", bufs=2)
                    for cc in range(CC):
                        nc.tensor.matmul(
                            vps[:, 0:256],
                            latT_sb[:, cc, kc0 * KC:(kc0 + 1) * KC],
                            wvu_sb[:, cc, :],
                            start=(cc == 0), stop=(cc == CC - 1))
                    for cc in range(CC):
                        nc.tensor.matmul(
                            vps[:, 256:512],
                            latT_sb[:, cc, (kc0 + 1) * KC:(kc0 + 2) * KC],
                            wvu_sb[:, cc, :],
                            start=(cc == 0), stop=(cc == CC - 1))
                    nc.vector.tensor_copy(
                        v_sb[:, kc0:kc0 + 2, :, 0:DH],
                        vps.rearrange("p (k h d) -> p k h d", k=2, h=HL))

            # =================== attention ===================
            for h in range(HL):
                for qg in range(NTG):
                    qs = slice(qg * TG, (qg + 1) * TG)
                    nkc = 4 * (qg + 1)
                    yps = psp.tile([128, TG], F32, name="yps", tag="Y", bufs=2)
                    # groups of up to 3 chunks share one S tile
                    groups = [list(range(g0, min(g0 + 3, nkc)))
                              for g0 in range(0, nkc, 3)]
                    pts = {}   # group idx -> (pt tile, chunk list)
                    prev_pv = None

                    def emit_pv(gi):
                        ptt, chunks = pts[gi]
                        for li, c in enumerate(chunks):
                            nc.tensor.matmul(
                                yps[0:DH + 1, :], v_sb[:, c, h, :],
                                ptt[:, li, :],
                                start=(c == 0), stop=(c == nkc - 1))

                    for gi, chunks in enumerate(groups):
                        sps = psp.tile(
                            [128, 3 * TG], F32, name="sps", tag="S", bufs=2)
                        for li, c in enumerate(chunks):
                            nc.tensor.matmul(
                                sps[:, li * TG:(li + 1) * TG],
                                kT[h][:, c * KC:(c + 1) * KC], qT[h][:, qs])
                        if prev_pv is not None:
                            emit_pv(prev_pv)
                        ptt = pool.tile(
                            [128, 3, TG], BF16, name="pt", tag="pt", bufs=3)
                        pts[gi] = (ptt, chunks)
                        # exp: wide over off-diagonal runs, granular on diag
                        li = 0
                        while li < len(chunks):
                            c = chunks[li]
                            j = c - (nkc - 4)
                            if j < 0:
                                l1 = li
                                while l1 < len(chunks) and chunks[l1] - (nkc - 4) < 0:
                                    l1 += 1
                                nc.scalar.activation(
                                    ptt[:, li:l1, :],
                                    sps[:, li * TG:l1 * TG].rearrange(
                                        "p (k t) -> p k t", k=l1 - li),
                                    AF.Exp, scale=SCALE)
                                li = l1
                            else:
                                if j > 0:
                                    nc.vector.memset(
                                        ptt[:, li, 0:j * KC], 0.0)
                                nc.scalar.activation(
                                    ptt[:, li, j * KC:TG],
                                    sps[:, li * TG + j * KC:(li + 1) * TG],
                                    AF.Exp, scale=SCALE)
                                nc.gpsimd.tensor_mul(
                                    ptt[:, li, j * KC:(j + 1) * KC],
                                    ptt[:, li, j * KC:(j + 1) * KC], tri_sb)
                                li += 1
                        # running per-partition max across chunks
                        for li, c in enumerate(chunks):
                            if c == 0:
                                nc.vector.tensor_copy(
                                    comb[:, qs], ptt[:, li, :])
                            else:
                                nc.vector.tensor_max(
                                    comb[:, qs], comb[:, qs], ptt[:, li, :])
                        prev_pv = gi
                    emit_pv(prev_pv)

                    # partition-max via 32x32 transpose + 2 reduces
                    hp = 64 * (h // 2) + h % 2
                    combT = pool.tile(
                        [128, TG], BF16, name="combT", tag="combT", bufs=2)
                    nc.vector.transpose(combT, comb[:, qs])
                    red = pool.tile(
                        [128, TG // 32], F32, name="red", tag="red", bufs=2)
                    nc.vector.reduce_max(
                        red, combT.rearrange("p (b j) -> p b j", j=32),
                        axis=AX.X)
                    stkt = pool.tile(
                        [32, 4, TG // 32], F32, name="stkt", tag="stkt", bufs=2)
                    for a in range(4):
                        nc.sync.dma_start(
                            out=stkt[:, a, :], in_=red[a * 32:(a + 1) * 32, :])
                    emf = pool.tile(
                        [32, TG // 32], F32, name="emf", tag="emf", bufs=2)
                    nc.vector.reduce_max(
                        emf, stkt.rearrange("p a b -> p b a"), axis=AX.X)
                    nc.sync.dma_start(
                        out=emst_sb[hp:hp + 1, qs].rearrange(
                            "p (i b) -> p i b", i=32),
                        in_=emf)

                    # stage y + D, scatter to yT2 / dsum
                    st65 = pool.tile(
                        [DH + 1, TG], F32, name="st65", tag="st65", bufs=3)
                    nc.vector.tensor_copy(st65, yps[0:DH + 1, :])
                    nc.gpsimd.dma_start(
                        out=yT2[(h % 2) * DH:(h % 2 + 1) * DH, h // 2, qs],
                        in_=st65[0:DH, :])
                    nc.sync.dma_start(
                        out=dsum_sb[hp:hp + 1, qs], in_=st65[DH:DH + 1, :])

            # =================== denominators + normalize ===================
            # un-permute the i-major per-block maxes into natural q order
            nc.vector.tensor_copy(
                emax_sb.rearrange("p (g b i) -> p g i b", b=16, i=32),
                emst_sb.rearrange("p (g i b) -> p g i b", i=32, b=16))
            # d = dsum + C * emax   (in place into dsum), r = 1/d (into emax)
            nc.vector.scalar_tensor_tensor(
                out=dsum_sb, in0=emax_sb, scalar=c_sb, in1=dsum_sb,
                op0=ALU.mult, op1=ALU.add)
            nc.vector.reciprocal(emax_sb, dsum_sb)
            r_bf = singles.tile([66, T], BF16)
            nc.vector.tensor_copy(r_bf, emax_sb)

            for g in range(2):
                for tg in range(NTG):
                    ts = slice(tg * TG, (tg + 1) * TG)
                    bcps = psp.tile([128, TG], F32, name="bc", tag="Y", bufs=2)
                    nc.tensor.matmul(
                        bcps, sel_sb[64 * g:64 * g + 2, :],
                        r_bf[64 * g:64 * g + 2, ts])
                    nc.vector.tensor_mul(
                        yT2[:, g, ts], yT2[:, g, ts], bcps)

            # =================== output projection ===================
            for tt in range(NKC):
                for eg in range(2):
                    ops = psp.tile([128, TG], F32, name="ops", tag="Y", bufs=2)
                    for fc in range(2):
                        nc.tensor.matmul(
                            ops, yT2[:, fc, tt * KC:(tt + 1) * KC],
                            wo_sb[:, fc, eg * TG:(eg + 1) * TG],
                            start=(fc == 0), stop=(fc == 1))
                    ost = pool.tile([128, TG], F32, name="ost", tag="ost", bufs=3)
                    if (tt * 2 + eg) % 2 == 0:
                        nc.scalar.copy(ost, ops)
                    else:
                        nc.vector.tensor_copy(ost, ops)
                    nc.sync.dma_start(
                        out=out[tt * KC:(tt + 1) * KC, eg * TG:(eg + 1) * TG],
                        in_=ost)

    return nc


def _tri():
    x = np.arange(128)[:, None]
    y = np.arange(128)[None, :]
    return (x <= y).astype(np.float32)  # keep iff key row <= query col


def _sel():
    s = np.zeros((66, 128), dtype=np.float32)
    for base in (0, 64):
        s[base + 0, 0:64] = 1.0
        s[base + 1, 64:128] = 1.0
    return s


def _lobo66(lg):
    v = np.zeros((66, 1), dtype=np.float32)
    for h in range(HL):
        v[64 * (h // 2) + h % 2, 0] = lg[h]
    return v


def _rope_tables():
    half = DR // 2
    inv = 1.0 / (10000.0 ** (np.arange(half, dtype=np.float64) / half))
    ang = np.arange(T, dtype=np.float64)[:, None] * inv[None, :]  # (T, half)
    cos = np.cos(ang).T  # (half, T)
    sin = np.sin(ang).T
    cosk = np.concatenate([cos, cos], axis=0)                 # (32, T)
    sink = np.concatenate([-sin, sin], axis=0)
    cosq = np.tile(cosk, (HL, 1)).astype(np.float32)          # (128, T)
    sinq = np.tile(sink, (HL, 1)).astype(np.float32)
    return cosq, sinq


def kernel(x, Wq, Wqr, Wkr, Wkvd, Wku, Wvu, Wo, lobo_log):
    x = np.asarray(x, dtype=np.float32)
    Wq = np.asarray(Wq, dtype=np.float32)
    Wqr = np.asarray(Wqr, dtype=np.float32)
    Wkr = np.asarray(Wkr, dtype=np.float32)
    Wkvd = np.asarray(Wkvd, dtype=np.float32)
    Wku = np.asarray(Wku, dtype=np.float32)
    Wvu = np.asarray(Wvu, dtype=np.float32)
    Wo = np.asarray(Wo, dtype=np.float32)
    lobo_log = np.asarray(lobo_log, dtype=np.float32)

    if "nc" not in _CACHE:
        _CACHE["nc"] = _build_program()
    nc = _CACHE["nc"]

    cosq, sinq = _rope_tables()
    in_maps = []
    for core in range(8):
        b, g = core // NG, core % NG
        hs = slice(g * HL * DH, (g + 1) * HL * DH)
        rs = slice(g * HL * DR, (g + 1) * HL * DR)
        in_maps.append({
            "xT": np.ascontiguousarray(x[b].T),
            "wq": np.ascontiguousarray(Wq[:, hs]),
            "wqr": np.ascontiguousarray(Wqr[:, rs]),
            "wkr": Wkr,
            "wkvd": Wkvd,
            "wku": np.ascontiguousarray(Wku[:, hs]),
            "wvu": np.ascontiguousarray(Wvu[:, hs]),
            "wo": np.ascontiguousarray(Wo[hs, :]),
            "cosq": cosq, "sinq": sinq,
            "tri": _tri(), "sel": _sel(),
            "lobo": _lobo66(lobo_log[g * HL:(g + 1) * HL]),
        })

    trace = bool(os.environ.get("BASS_TRACE_KERNEL"))
    bkr = run_bass_kernel_spmd(
        nc, in_maps, core_ids=list(range(8)), trace=trace)
    if trace:
        print(f"HW exec time: {bkr.exec_time_ns} ns")
        if bkr.instructions_and_trace is not None:
            print("trace:", bkr.instructions_and_trace[1])
        _CACHE["last_result"] = bkr
    res = bkr.results
    out = np.zeros((B, T, E), dtype=np.float32)
    for core in range(8):
        out[core // NG] += res[core]["out"]
    return out


# revision 13
# speedup vs baseline: 1.4295x; 1.0320x over previous
"""MLA (multi-head latent attention) Bass kernel for 8 trn2 NeuronCores.

Sharding: core = b*4 + g  (b in {0,1} batches, g in {0..3} head-groups of 4 heads).
Each core: projections from xT (bf16 matmuls), flash-style causal attention with
k-major scores (S^T) so exp'd probs feed PV directly, LOBO softmax
attn = exp(s) / (sum_k exp(s) + C*exp(max_k s)), row-parallel out-proj partial.
Host sums the 4 partials per batch.

v2 layout notes:
  - PSUM tags: S = [128,1536] (3 banks) x2, Y = [128,512] x2  -> 8 banks total.
  - Projections pack (kv0,kv1,kr) and (q01,q23,qr) each into one S tile.
  - Attention: per (h,qg) score chunks land 3-per-S-tile; one wide exp per
    off-diagonal run, granular exps + DVE-memset + gpsimd triangle-mask on the
    4 diagonal chunks.  Per-query running max kept in a [128,T] bf16 comb tile
    (DVE tensor_max); partition-max via ONE gpsimd tensor_reduce(axis=C).
  - Denominator D rides a ones-column in V through the PV matmul (row 64).
  - y normalization: r broadcast to 64-row blocks via a tiny f32r matmul with a
    0/1 selector lhsT; yT2 multiplied in place.
"""

import math
import os

import ml_dtypes
import numpy as np

BF16NP = ml_dtypes.bfloat16

import concourse.bass as bass
import concourse.bass_isa as bass_isa
import concourse.mybir as mybir
import concourse.tile as _tile_mod
from concourse.tile import TileContext
from concourse.vector_clock import ScopedClock, VectorClock
import bass_rust as _bass_rust
from concourse.bass_utils import run_bass_kernel_spmd

_N_PROCS = _bass_rust.N_PROCS


def _split_drain_and_barrier(self, tick_clock, wait_clock):
    """Replacement for TileContext._drain_and_barrier: the stock version puts
    the whole global vector clock (up to 27 sem waits) on one Drain, which this
    walrus rejects ("Too many sync wait commands").  Emit one Drain per
    outstanding processor instead."""
    gc = tick_clock.global_clock
    procs = [p for p in range(_N_PROCS) if gc[p] > 0]
    for p in procs:
        vc = VectorClock([gc[q] if q == p else 0 for q in range(_N_PROCS)])
        d = self.nc.sync.drain()
        wait_clock.add_sem_waits(d.ins, ScopedClock({None: vc}))
    self.nc.all_engine_barrier()
    popped = self.nc._tile_sem_poison_stack.pop()
    assert popped is self._sem_poison
    self.nc.clear_and_free_semaphores(list(self.sems.allocated().values()))
    self.nc.all_engine_barrier()


_tile_mod.TileContext._drain_and_barrier = _split_drain_and_barrier

# ---------------------------------------------------------------------------
# This walrus build enforces small per-instruction sync-wait budgets
# ("Too many sync wait commands").  Post-process the BIR JSON: any
# instruction carrying more than its budget of waits gets the excess
# hoisted onto same-engine Drain carriers inserted immediately before it
# (same program point on the engine's sequential stream -> semantics
# unchanged).
# ---------------------------------------------------------------------------
_orig_to_json_bytes = bass.Bass.to_json_bytes
_WAIT_LIMITS = {"Drain": 1, "DMACopy": 1}
_DEF_WAIT_LIMIT = 1


def _to_json_split_waits(self, *a, **kw):
    import json as _json
    data = _json.loads(_orig_to_json_bytes(self, *a, **kw))
    nid = 0
    for f in data.get("functions", []):
        for bb in f.get("blocks", []):
            out = []
            for inst in bb.get("instructions", []):
                si = inst.get("sync_info")
                if isinstance(si, dict):
                    w = si.get("on_wait")
                    if isinstance(w, list):
                        k = _WAIT_LIMITS.get(inst.get("opcode"), _DEF_WAIT_LIMIT)
                        if len(w) > k:
                            extra, keep = w[:-k], w[-k:]
                            for wt in extra:
                                out.append({
                                    "debug": inst.get("debug"),
                                    "engine": inst["engine"],
                                    "ins": [], "outs": [],
                                    "name": f"wsplit-{nid}",
                                    "opcode": "Drain",
                                    "sync_info": {"on_update": [],
                                                  "on_wait": [wt]},
                                })
                                nid += 1
                            si["on_wait"] = keep
                out.append(inst)
            bb["instructions"] = out
    return _json.dumps(data).encode()


bass.Bass.to_json_bytes = _to_json_split_waits

B, T, E = 2, 2048, 1024
H, DH = 16, 64
DKV = 256
DR = 32
HL = 4              # heads per core
NG = 4              # head groups
SCALE = 1.0 / math.sqrt(DH + DR)
TG = 512            # query-group width
KC = 128            # key-chunk width
NTG = T // TG       # 4
NKC = T // KC       # 16
EC = E // 128       # 8  e-chunks
CC = DKV // 128     # 2  latent chunks

F32 = mybir.dt.float32
F32R = mybir.dt.float32r
BF16 = mybir.dt.bfloat16
AF = mybir.ActivationFunctionType
ALU = mybir.AluOpType
AX = mybir.AxisListType

_CACHE = {}


def _r(ap):
    return ap.bitcast(F32R)


def _build_program():
    nc = bass.Bass()

    xT = nc.declare_dram_parameter("xT", [E, T], BF16, isOutput=False)
    wq = nc.declare_dram_parameter("wq", [E, HL * DH], BF16, isOutput=False)
    wqr = nc.declare_dram_parameter("wqr", [E, HL * DR], BF16, isOutput=False)
    wkr = nc.declare_dram_parameter("wkr", [E, DR], BF16, isOutput=False)
    wkvd = nc.declare_dram_parameter("wkvd", [E, DKV], BF16, isOutput=False)
    wku = nc.declare_dram_parameter("wku", [DKV, HL * DH], BF16, isOutput=False)
    wvu = nc.declare_dram_parameter("wvu", [DKV, HL * DH], BF16, isOutput=False)
    wo = nc.declare_dram_parameter("wo", [HL * DH, E], BF16, isOutput=False)
    cosq = nc.declare_dram_parameter("cosq", [HL * DR, T], BF16, isOutput=False)
    sinq = nc.declare_dram_parameter("sinq", [HL * DR, T], BF16, isOutput=False)
    tri = nc.declare_dram_parameter("tri", [128, 128], BF16, isOutput=False)
    sel = nc.declare_dram_parameter("sel", [66, 128], BF16, isOutput=False)
    lobo = nc.declare_dram_parameter("lobo", [66, 1], F32, isOutput=False)
    out = nc.declare_dram_parameter("out", [T, E], F32, isOutput=True)

    with TileContext(nc) as tc:
        from contextlib import ExitStack

        with ExitStack() as ctx:
            singles = ctx.enter_context(tc.tile_pool(name="singles", bufs=1))
            pool = ctx.enter_context(tc.tile_pool(name="pool", bufs=2))
            psp = ctx.enter_context(tc.tile_pool(name="psp", bufs=1, space="PSUM"))

            # ---------------- weights (bf16 in SBUF) ----------------
            wq_sb = singles.tile([128, EC, HL * DH], BF16)
            nc.sync.dma_start(
                out=wq_sb, in_=wq.rearrange("(c p) f -> p c f", p=128))
            wqr_sb = singles.tile([128, EC, HL * DR], BF16)
            nc.gpsimd.dma_start(
                out=wqr_sb, in_=wqr.rearrange("(c p) f -> p c f", p=128))
            wkr_sb = singles.tile([128, EC, DR], BF16)
            nc.sync.dma_start(
                out=wkr_sb, in_=wkr.rearrange("(c p) f -> p c f", p=128))
            wkvd_sb = singles.tile([128, EC, DKV], BF16)
            nc.gpsimd.dma_start(
                out=wkvd_sb, in_=wkvd.rearrange("(c p) f -> p c f", p=128))
            wku_sb = singles.tile([128, CC, HL * DH], BF16)
            nc.sync.dma_start(
                out=wku_sb, in_=wku.rearrange("(c p) f -> p c f", p=128))
            wvu_sb = singles.tile([128, CC, HL * DH], BF16)
            nc.gpsimd.dma_start(
                out=wvu_sb, in_=wvu.rearrange("(c p) f -> p c f", p=128))
            wo_sb = singles.tile([128, 2, E], BF16)
            nc.gpsimd.dma_start(
                out=wo_sb, in_=wo.rearrange("(c p) e -> p c e", p=128))

            cosq_sb = singles.tile([128, T], BF16)
            nc.sync.dma_start(out=cosq_sb, in_=cosq[:, :])
            sinq_sb = singles.tile([128, T], BF16)
            nc.sync.dma_start(out=sinq_sb, in_=sinq[:, :])
            tri_sb = singles.tile([128, 128], BF16)
            nc.sync.dma_start(out=tri_sb, in_=tri[:, :])
            sel_sb = singles.tile([66, 128], BF16)
            nc.sync.dma_start(out=sel_sb, in_=sel[:, :])
            lobo_sb = singles.tile([66, 1], F32)
            nc.sync.dma_start(out=lobo_sb, in_=lobo[:, :])
            c_sb = singles.tile([66, 1], F32)
            nc.scalar.activation(c_sb, lobo_sb, AF.Exp)

            # ---------------- persistent activation tiles ----------------
            xt_sb = singles.tile([128, EC, T], BF16)
            xT_r = xT.rearrange("(c p) t -> p c t", p=128)
            latT_sb = singles.tile([128, CC, T], BF16)
            qT = [singles.tile([96, T], BF16, name=f"qT{h}") for h in range(HL)]
            kT = [singles.tile([96, T], BF16, name=f"kT{h}") for h in range(HL)]
            v_sb = singles.tile([128, NKC, HL, DH + 1], BF16)
            nc.vector.memset(v_sb[:, :, :, DH:DH + 1], 1.0)
            yT2 = singles.tile([128, 2, T], BF16)
            comb = singles.tile([128, T], BF16)
            # head h row lives at partition 64*(h//2) + h%2
            dsum_sb = singles.tile([66, T], F32)
            emax_sb = singles.tile([66, T], F32)
            emst_sb = singles.tile([66, T], F32)
            nc.vector.memset(dsum_sb, 1.0)
            nc.vector.memset(emax_sb, 1.0)
            nc.vector.memset(emst_sb, 1.0)

            # =================== projections, per tg ===================
            for tg in range(NTG):
                ts = slice(tg * TG, (tg + 1) * TG)
                nc.gpsimd.dma_start(
                    out=xt_sb[:, :, ts], in_=xT_r[:, :, ts])
                xts = [xt_sb[:, ec, ts] for ec in range(EC)]

                # --- latent (kv) halves + k_rope into one S tile ---
                skv = psp.tile([128, 3 * TG], F32, name="skv", tag="S", bufs=2)
                for ec in range(EC):
                    nc.tensor.matmul(
                        skv[:, 0:TG], wkvd_sb[:, ec, 0:128], xts[ec],
                        start=(ec == 0), stop=(ec == EC - 1))
                for ec in range(EC):
                    nc.tensor.matmul(
                        skv[:, TG:2 * TG], wkvd_sb[:, ec, 128:256], xts[ec],
                        start=(ec == 0), stop=(ec == EC - 1))
                for ec in range(EC):
                    nc.tensor.matmul(
                        skv[0:DR, 2 * TG:3 * TG], wkr_sb[:, ec, :], xts[ec],
                        start=(ec == 0), stop=(ec == EC - 1))
                nc.scalar.copy(
                    latT_sb[:, :, ts],
                    skv[:, 0:2 * TG].rearrange("p (c t) -> p c t", c=2))
                kr_pre = pool.tile([DR, TG], BF16, name="krp", tag="krp", bufs=2)
                nc.scalar.copy(kr_pre, skv[0:DR, 2 * TG:3 * TG])

                # k_rope rotate-half + tables
                kr_sw = pool.tile([DR, TG], BF16, name="krs", tag="krs", bufs=2)
                nc.sync.dma_start(out=kr_sw[0:16, :], in_=kr_pre[16:32, :])
                nc.sync.dma_start(out=kr_sw[16:32, :], in_=kr_pre[0:16, :])
                kr_m = pool.tile([DR, TG], BF16, name="krm", tag="krm", bufs=2)
                nc.vector.tensor_mul(kr_m, kr_pre, cosq_sb[0:DR, ts])
                nc.vector.tensor_mul(kr_sw, kr_sw, sinq_sb[0:DR, ts])
                nc.vector.tensor_add(kr_m, kr_m, kr_sw)
                for h in range(HL):
                    nc.sync.dma_start(out=kT[h][DH:96, ts], in_=kr_m)

                # --- q halves + q_rope into one S tile ---
                sq = psp.tile([128, 3 * TG], F32, name="sq", tag="S", bufs=2)
                for ec in range(EC):
                    nc.tensor.matmul(
                        sq[:, 0:TG], wq_sb[:, ec, 0:128], xts[ec],
                        start=(ec == 0), stop=(ec == EC - 1))
                for ec in range(EC):
                    nc.tensor.matmul(
                        sq[:, TG:2 * TG], wq_sb[:, ec, 128:256], xts[ec],
                        start=(ec == 0), stop=(ec == EC - 1))
                for ec in range(EC):
                    nc.tensor.matmul(
                        sq[:, 2 * TG:3 * TG], wqr_sb[:, ec, :], xts[ec],
                        start=(ec == 0), stop=(ec == EC - 1))
                stq = pool.tile([128, 2 * TG], BF16, name="stq", tag="stq", bufs=2)
                nc.scalar.copy(stq, sq[:, 0:2 * TG])
                nc.sync.dma_start(out=qT[0][0:DH, ts], in_=stq[0:DH, 0:TG])
                nc.sync.dma_start(out=qT[1][0:DH, ts], in_=stq[DH:128, 0:TG])
                nc.sync.dma_start(out=qT[2][0:DH, ts], in_=stq[0:DH, TG:2 * TG])
                nc.sync.dma_start(out=qT[3][0:DH, ts], in_=stq[DH:128, TG:2 * TG])
                rp_pre = pool.tile([128, TG], BF16, name="rpp", tag="rpp", bufs=2)
                nc.scalar.copy(rp_pre, sq[:, 2 * TG:3 * TG])
                rp_sw = pool.tile([128, TG], BF16, name="rps", tag="rps", bufs=2)
                for h in range(HL):
                    nc.sync.dma_start(
                        out=rp_sw[h * DR:h * DR + 16, :],
                        in_=rp_pre[h * DR + 16:h * DR + 32, :])
                    nc.sync.dma_start(
                        out=rp_sw[h * DR + 16:h * DR + 32, :],
                        in_=rp_pre[h * DR:h * DR + 16, :])
                rp_m = pool.tile([128, TG], BF16, name="rpm", tag="rpm", bufs=2)
                nc.vector.tensor_mul(rp_m, rp_pre, cosq_sb[:, ts])
                nc.vector.tensor_mul(rp_sw, rp_sw, sinq_sb[:, ts])
                nc.vector.tensor_add(rp_m, rp_m, rp_sw)
                for h in range(HL):
                    nc.sync.dma_start(
                        out=qT[h][DH:96, ts], in_=rp_m[h * DR:(h + 1) * DR, :])

                # --- k_c from latent ---
                skc = psp.tile([128, 3 * TG], F32, name="skc", tag="S", bufs=2)
                for cc in range(CC):
                    nc.tensor.matmul(
                        skc[:, 0:TG], wku_sb[:, cc, 0:128], latT_sb[:, cc, ts],
                        start=(cc == 0), stop=(cc == CC - 1))
                for cc in range(CC):
                    nc.tensor.matmul(
                        skc[:, TG:2 * TG], wku_sb[:, cc, 128:256],
                        latT_sb[:, cc, ts],
                        start=(cc == 0), stop=(cc == CC - 1))
                stk = pool.tile([128, 2 * TG], BF16, name="stk", tag="stq", bufs=2)
                nc.scalar.copy(stk, skc[:, 0:2 * TG])
                nc.sync.dma_start(out=kT[0][0:DH, ts], in_=stk[0:DH, 0:TG])
                nc.sync.dma_start(out=kT[1][0:DH, ts], in_=stk[DH:128, 0:TG])
                nc.sync.dma_start(out=kT[2][0:DH, ts], in_=stk[0:DH, TG:2 * TG])
                nc.sync.dma_start(out=kT[3][0:DH, ts], in_=stk[DH:128, TG:2 * TG])

                # --- V (natural layout) for this tg's 4 key chunks ---
                for half in range(2):
                    kc0 = 4 * tg + 2 * half
                    vps = psp.tile([128, TG], F32, name="vps", tag="Y", bufs=2)
                    for cc in range(CC):
                        nc.tensor.matmul(
                            vps[:, 0:256],
                            latT_sb[:, cc, kc0 * KC:(kc0 + 1) * KC],
                            wvu_sb[:, cc, :],
                            start=(cc == 0), stop=(cc == CC - 1))
                    for cc in range(CC):
                        nc.tensor.matmul(
                            vps[:, 256:512],
                            latT_sb[:, cc, (kc0 + 1) * KC:(kc0 + 2) * KC],
                            wvu_sb[:, cc, :],
                            start=(cc == 0), stop=(cc == CC - 1))
                    nc.scalar.copy(
                        v_sb[:, kc0:kc0 + 2, :, 0:DH],
                        vps.rearrange("p (k h d) -> p k h d", k=2, h=HL))

            # =================== attention ===================
            for h in range(HL):
                for qg in range(NTG):
                    qs = slice(qg * TG, (qg + 1) * TG)
                    nkc = 4 * (qg + 1)
                    yps = psp.tile([128, TG], F32, name="yps", tag="Y", bufs=2)
                    # groups of up to 3 chunks share one S tile
                    groups = [list(range(g0, min(g0 + 3, nkc)))
                              for g0 in range(0, nkc, 3)]
                    pts = {}   # group idx -> (pt tile, chunk list)
                    prev_pv = None

                    def joff(c):
                        # valid columns of chunk c start at j*KC (diag rounding)
                        j = c - (nkc - 4)
                        return j * KC if j > 0 else 0

                    def emit_pv(gi):
                        ptt, chunks = pts[gi]
                        for li, c in enumerate(chunks):
                            o = joff(c)
                            nc.tensor.matmul(
                                yps[0:DH + 1, o:TG], v_sb[:, c, h, :],
                                ptt[:, li, o:TG],
                                start=(c == 0), stop=(c == nkc - 1),
                                skip_group_check=True)

                    for gi, chunks in enumerate(groups):
                        sps = psp.tile(
                            [128, 3 * TG], F32, name="sps", tag="S", bufs=2)
                        for li, c in enumerate(chunks):
                            o = joff(c)
                            nc.tensor.matmul(
                                sps[:, li * TG + o:(li + 1) * TG],
                                kT[h][:, c * KC:(c + 1) * KC],
                                qT[h][:, qg * TG + o:(qg + 1) * TG])
                        if prev_pv is not None:
                            emit_pv(prev_pv)
                        ptt = pool.tile(
                            [128, 3, TG], BF16, name="pt", tag="pt", bufs=3)
                        pts[gi] = (ptt, chunks)
                        # exp: wide over off-diagonal runs, granular on diag
                        li = 0
                        while li < len(chunks):
                            c = chunks[li]
                            j = c - (nkc - 4)
                            if j < 0:
                                l1 = li
                                while l1 < len(chunks) and chunks[l1] - (nkc - 4) < 0:
                                    l1 += 1
                                nc.scalar.activation(
                                    ptt[:, li:l1, :],
                                    sps[:, li * TG:l1 * TG].rearrange(
                                        "p (k t) -> p k t", k=l1 - li),
                                    AF.Exp, scale=SCALE)
                                li = l1
                            else:
                                o = j * KC
                                nc.scalar.activation(
                                    ptt[:, li, o:TG],
                                    sps[:, li * TG + o:(li + 1) * TG],
                                    AF.Exp, scale=SCALE)
                                nc.gpsimd.tensor_mul(
                                    ptt[:, li, o:o + KC],
                                    ptt[:, li, o:o + KC], tri_sb)
                                li += 1
                        # running per-partition max across chunks
                        for li, c in enumerate(chunks):
                            o = joff(c)
                            if c == 0:
                                nc.vector.tensor_copy(
                                    comb[:, qs], ptt[:, li, :])
                            else:
                                nc.vector.tensor_max(
                                    comb[:, qg * TG + o:(qg + 1) * TG],
                                    comb[:, qg * TG + o:(qg + 1) * TG],
                                    ptt[:, li, o:TG])
                        prev_pv = gi
                    emit_pv(prev_pv)

                    # stage y + D, scatter to yT2 / dsum
                    hp = 64 * (h // 2) + h % 2
                    st65 = pool.tile(
                        [DH + 1, TG], F32, name="st65", tag="st65", bufs=3)
                    nc.vector.tensor_copy(st65, yps[0:DH + 1, :])
                    nc.gpsimd.dma_start(
                        out=yT2[(h % 2) * DH:(h % 2 + 1) * DH, h // 2, qs],
                        in_=st65[0:DH, :])
                    nc.sync.dma_start(
                        out=dsum_sb[hp:hp + 1, qs], in_=st65[DH:DH + 1, :])

                # per-head partition-max via 32x32 transpose + 2 reduces
                combT = pool.tile(
                    [128, T], BF16, name="combT", tag="combT", bufs=2)
                nc.vector.transpose(combT, comb)
                red = pool.tile(
                    [128, T // 32], F32, name="red", tag="red", bufs=2)
                nc.vector.reduce_max(
                    red, combT.rearrange("p (b j) -> p b j", j=32),
                    axis=AX.X)
                stkt = pool.tile(
                    [32, 4, T // 32], F32, name="stkt", tag="stkt", bufs=2)
                for a in range(4):
                    nc.sync.dma_start(
                        out=stkt[:, a, :], in_=red[a * 32:(a + 1) * 32, :])
                emf = pool.tile(
                    [32, T // 32], F32, name="emf", tag="emf", bufs=2)
                nc.vector.reduce_max(
                    emf, stkt.rearrange("p a b -> p b a"), axis=AX.X)
                nc.sync.dma_start(
                    out=emst_sb[hp:hp + 1, :].rearrange(
                        "p (i b) -> p i b", i=32),
                    in_=emf)

            # ========== denominators + normalize + out-proj, per tg ==========
            r_bf = singles.tile([66, T], BF16)
            emst_r = emst_sb.rearrange("p (i b) -> p i b", b=64)
            for tg in range(NTG):
                ts = slice(tg * TG, (tg + 1) * TG)
                # un-permute this chunk of the i-major per-block maxes
                nc.vector.tensor_copy(
                    emax_sb[:, ts].rearrange("p (b i) -> p i b", i=32),
                    emst_r[:, :, 16 * tg:16 * (tg + 1)])
                # d = dsum + C * emax (in place), r = 1/d (into emax)
                nc.vector.scalar_tensor_tensor(
                    out=dsum_sb[:, ts], in0=emax_sb[:, ts], scalar=c_sb,
                    in1=dsum_sb[:, ts], op0=ALU.mult, op1=ALU.add)
                nc.vector.reciprocal(emax_sb[:, ts], dsum_sb[:, ts])
                nc.vector.tensor_copy(r_bf[:, ts], emax_sb[:, ts])
                for g in range(2):
                    bcps = psp.tile([128, TG], F32, name="bc", tag="Y", bufs=2)
                    nc.tensor.matmul(
                        bcps, sel_sb[64 * g:64 * g + 2, :],
                        r_bf[64 * g:64 * g + 2, ts])
                    nc.vector.tensor_mul(
                        yT2[:, g, ts], yT2[:, g, ts], bcps)
                for tt in range(4 * tg, 4 * (tg + 1)):
                    for eg in range(2):
                        ops = psp.tile(
                            [128, TG], F32, name="ops", tag="Y", bufs=2)
                        for fc in range(2):
                            nc.tensor.matmul(
                                ops, yT2[:, fc, tt * KC:(tt + 1) * KC],
                                wo_sb[:, fc, eg * TG:(eg + 1) * TG],
                                start=(fc == 0), stop=(fc == 1))
                        ost = pool.tile(
                            [128, TG], F32, name="ost", tag="ost", bufs=3)
                        if (tt * 2 + eg) % 2 == 0:
                            nc.scalar.copy(ost, ops)
                        else:
                            nc.vector.tensor_copy(ost, ops)
                        nc.sync.dma_start(
                            out=out[tt * KC:(tt + 1) * KC,
                                    eg * TG:(eg + 1) * TG],
                            in_=ost)

    return nc


def _tri():
    x = np.arange(128)[:, None]
    y = np.arange(128)[None, :]
    return (x <= y).astype(np.float32)  # keep iff key row <= query col


def _sel():
    s = np.zeros((66, 128), dtype=np.float32)
    for base in (0, 64):
        s[base + 0, 0:64] = 1.0
        s[base + 1, 64:128] = 1.0
    return s


def _lobo66(lg):
    v = np.zeros((66, 1), dtype=np.float32)
    for h in range(HL):
        v[64 * (h // 2) + h % 2, 0] = lg[h]
    return v


def _rope_tables():
    half = DR // 2
    inv = 1.0 / (10000.0 ** (np.arange(half, dtype=np.float64) / half))
    ang = np.arange(T, dtype=np.float64)[:, None] * inv[None, :]  # (T, half)
    cos = np.cos(ang).T  # (half, T)
    sin = np.sin(ang).T
    cosk = np.concatenate([cos, cos], axis=0)                 # (32, T)
    sink = np.concatenate([-sin, sin], axis=0)
    cosq = np.tile(cosk, (HL, 1)).astype(np.float32)          # (128, T)
    sinq = np.tile(sink, (HL, 1)).astype(np.float32)
    return cosq, sinq


def kernel(x, Wq, Wqr, Wkr, Wkvd, Wku, Wvu, Wo, lobo_log):
    x = np.asarray(x, dtype=np.float32)
    Wq = np.asarray(Wq, dtype=np.float32)
    Wqr = np.asarray(Wqr, dtype=np.float32)
    Wkr = np.asarray(Wkr, dtype=np.float32)
    Wkvd = np.asarray(Wkvd, dtype=np.float32)
    Wku = np.asarray(Wku, dtype=np.float32)
    Wvu = np.asarray(Wvu, dtype=np.float32)
    Wo = np.asarray(Wo, dtype=np.float32)
    lobo_log = np.asarray(lobo_log, dtype=np.float32)

    if "nc" not in _CACHE:
        _CACHE["nc"] = _build_program()
    nc = _CACHE["nc"]

    cosq, sinq = _rope_tables()
    bf = lambda a: np.ascontiguousarray(a).astype(BF16NP)
    xTb = [bf(x[b].T) for b in range(B)]
    wkr_b, wkvd_b = bf(Wkr), bf(Wkvd)
    cosq_b, sinq_b = bf(cosq), bf(sinq)
    tri_b, sel_b = bf(_tri()), bf(_sel())
    in_maps = []
    for core in range(8):
        b, g = core // NG, core % NG
        hs = slice(g * HL * DH, (g + 1) * HL * DH)
        rs = slice(g * HL * DR, (g + 1) * HL * DR)
        in_maps.append({
            "xT": xTb[b],
            "wq": bf(Wq[:, hs]),
            "wqr": bf(Wqr[:, rs]),
            "wkr": wkr_b,
            "wkvd": wkvd_b,
            "wku": bf(Wku[:, hs]),
            "wvu": bf(Wvu[:, hs]),
            "wo": bf(Wo[hs, :]),
            "cosq": cosq_b, "sinq": sinq_b,
            "tri": tri_b, "sel": sel_b,
            "lobo": _lobo66(lobo_log[g * HL:(g + 1) * HL]),
        })

    trace = bool(os.environ.get("BASS_TRACE_KERNEL"))
    bkr = run_bass_kernel_spmd(
        nc, in_maps, core_ids=list(range(8)), trace=trace)
    if trace:
        print(f"HW exec time: {bkr.exec_time_ns} ns")
        if bkr.instructions_and_trace is not None:
            print("trace:", bkr.instructions_and_trace[1])
        _CACHE["last_result"] = bkr
    res = bkr.results
    out = np.zeros((B, T, E), dtype=np.float32)
    for core in range(8):
        out[core // NG] += res[core]["out"]
    return out


# revision 17
# speedup vs baseline: 1.6169x; 1.1311x over previous
"""MLA (multi-head latent attention) Bass kernel for 8 trn2 NeuronCores.

Sharding: core = b*4 + g  (b in {0,1} batches, g in {0..3} head-groups of 4 heads).
Each core: projections from xT (bf16 matmuls), flash-style causal attention with
k-major scores (S^T) so exp'd probs feed PV directly, LOBO softmax
attn = exp(s) / (sum_k exp(s) + C*exp(max_k s)), row-parallel out-proj partial.
Host sums the 4 partials per batch.

v2 layout notes:
  - PSUM tags: S = [128,1536] (3 banks) x2, Y = [128,512] x2  -> 8 banks total.
  - Projections pack (kv0,kv1,kr) and (q01,q23,qr) each into one S tile.
  - Attention: per (h,qg) score chunks land 3-per-S-tile; one wide exp per
    off-diagonal run, granular exps + DVE-memset + gpsimd triangle-mask on the
    4 diagonal chunks.  Per-query running max kept in a [128,T] bf16 comb tile
    (DVE tensor_max); partition-max via ONE gpsimd tensor_reduce(axis=C).
  - Denominator D rides a ones-column in V through the PV matmul (row 64).
  - y normalization: r broadcast to 64-row blocks via a tiny f32r matmul with a
    0/1 selector lhsT; yT2 multiplied in place.
"""

import math
import os

import ml_dtypes
import numpy as np

BF16NP = ml_dtypes.bfloat16

import concourse.bass as bass
import concourse.bass_isa as bass_isa
import concourse.mybir as mybir
import concourse.tile as _tile_mod
from concourse.tile import TileContext
from concourse.vector_clock import ScopedClock, VectorClock
import bass_rust as _bass_rust
from concourse.bass_utils import run_bass_kernel_spmd

_N_PROCS = _bass_rust.N_PROCS


def _split_drain_and_barrier(self, tick_clock, wait_clock):
    """Replacement for TileContext._drain_and_barrier: the stock version puts
    the whole global vector clock (up to 27 sem waits) on one Drain, which this
    walrus rejects ("Too many sync wait commands").  Emit one Drain per
    outstanding processor instead."""
    gc = tick_clock.global_clock
    procs = [p for p in range(_N_PROCS) if gc[p] > 0]
    for p in procs:
        vc = VectorClock([gc[q] if q == p else 0 for q in range(_N_PROCS)])
        d = self.nc.sync.drain()
        wait_clock.add_sem_waits(d.ins, ScopedClock({None: vc}))
    self.nc.all_engine_barrier()
    popped = self.nc._tile_sem_poison_stack.pop()
    assert popped is self._sem_poison
    self.nc.clear_and_free_semaphores(list(self.sems.allocated().values()))
    self.nc.all_engine_barrier()


_tile_mod.TileContext._drain_and_barrier = _split_drain_and_barrier

# ---------------------------------------------------------------------------
# This walrus build enforces small per-instruction sync-wait budgets
# ("Too many sync wait commands").  Post-process the BIR JSON: any
# instruction carrying more than its budget of waits gets the excess
# hoisted onto same-engine Drain carriers inserted immediately before it
# (same program point on the engine's sequential stream -> semantics
# unchanged).
# ---------------------------------------------------------------------------
_orig_to_json_bytes = bass.Bass.to_json_bytes
_WAIT_LIMITS = {"Drain": 1, "DMACopy": 1}
_DEF_WAIT_LIMIT = 1


def _to_json_split_waits(self, *a, **kw):
    import json as _json
    data = _json.loads(_orig_to_json_bytes(self, *a, **kw))
    nid = 0
    for f in data.get("functions", []):
        for bb in f.get("blocks", []):
            out = []
            for inst in bb.get("instructions", []):
                si = inst.get("sync_info")
                if isinstance(si, dict):
                    w = si.get("on_wait")
                    if isinstance(w, list):
                        k = _WAIT_LIMITS.get(inst.get("opcode"), _DEF_WAIT_LIMIT)
                        if len(w) > k:
                            extra, keep = w[:-k], w[-k:]
                            for wt in extra:
                                out.append({
                                    "debug": inst.get("debug"),
                                    "engine": inst["engine"],
                                    "ins": [], "outs": [],
                                    "name": f"wsplit-{nid}",
                                    "opcode": "Drain",
                                    "sync_info": {"on_update": [],
                                                  "on_wait": [wt]},
                                })
                                nid += 1
                            si["on_wait"] = keep
                out.append(inst)
            bb["instructions"] = out
    return _json.dumps(data).encode()


bass.Bass.to_json_bytes = _to_json_split_waits

B, T, E = 2, 2048, 1024
H, DH = 16, 64
DKV = 256
DR = 32
HL = 4              # heads per core
NG = 4              # head groups
SCALE = 1.0 / math.sqrt(DH + DR)
TG = 512            # query-group width
KC = 128            # key-chunk width
NTG = T // TG       # 4
NKC = T // KC       # 16
EC = E // 128       # 8  e-chunks
CC = DKV // 128     # 2  latent chunks

F32 = mybir.dt.float32
F32R = mybir.dt.float32r
BF16 = mybir.dt.bfloat16
AF = mybir.ActivationFunctionType
ALU = mybir.AluOpType
AX = mybir.AxisListType

_CACHE = {}


def _r(ap):
    return ap.bitcast(F32R)


def _build_program():
    nc = bass.Bass()

    xT = nc.declare_dram_parameter("xT", [E, T], BF16, isOutput=False)
    wq = nc.declare_dram_parameter("wq", [E, HL * DH], BF16, isOutput=False)
    wqr = nc.declare_dram_parameter("wqr", [E, HL * DR], BF16, isOutput=False)
    wkr = nc.declare_dram_parameter("wkr", [E, DR], BF16, isOutput=False)
    wkvd = nc.declare_dram_parameter("wkvd", [E, DKV], BF16, isOutput=False)
    wku = nc.declare_dram_parameter("wku", [DKV, HL * DH], BF16, isOutput=False)
    wvu = nc.declare_dram_parameter("wvu", [DKV, HL * DH], BF16, isOutput=False)
    wo = nc.declare_dram_parameter("wo", [HL * DH, E], BF16, isOutput=False)
    cosq = nc.declare_dram_parameter("cosq", [HL * DR, T], BF16, isOutput=False)
    sinq = nc.declare_dram_parameter("sinq", [HL * DR, T], BF16, isOutput=False)
    tri = nc.declare_dram_parameter("tri", [128, 128], BF16, isOutput=False)
    sel = nc.declare_dram_parameter("sel", [66, 128], BF16, isOutput=False)
    lobo = nc.declare_dram_parameter("lobo", [66, 1], F32, isOutput=False)
    out = nc.declare_dram_parameter("out", [T, E], BF16, isOutput=True)

    with TileContext(nc) as tc:
        from contextlib import ExitStack

        with ExitStack() as ctx:
            singles = ctx.enter_context(tc.tile_pool(name="singles", bufs=1))
            pool = ctx.enter_context(tc.tile_pool(name="pool", bufs=2))
            psp = ctx.enter_context(tc.tile_pool(name="psp", bufs=1, space="PSUM"))

            # ---------------- weights (bf16 in SBUF) ----------------
            # tg0-critical loads lead each queue; wo is deferred to the end
            wq_sb = singles.tile([128, EC, HL * DH], BF16)
            nc.sync.dma_start(
                out=wq_sb, in_=wq.rearrange("(c p) f -> p c f", p=128))
            wkr_sb = singles.tile([128, EC, DR], BF16)
            nc.sync.dma_start(
                out=wkr_sb, in_=wkr.rearrange("(c p) f -> p c f", p=128))
            wkvd_sb = singles.tile([128, EC, DKV], BF16)
            nc.gpsimd.dma_start(
                out=wkvd_sb, in_=wkvd.rearrange("(c p) f -> p c f", p=128))
            wqr_sb = singles.tile([128, EC, HL * DR], BF16)
            nc.gpsimd.dma_start(
                out=wqr_sb, in_=wqr.rearrange("(c p) f -> p c f", p=128))
            wku_sb = singles.tile([128, CC, HL * DH], BF16)
            nc.gpsimd.dma_start(
                out=wku_sb, in_=wku.rearrange("(c p) f -> p c f", p=128))
            wvu_sb = singles.tile([128, CC, HL * DH], BF16)
            nc.gpsimd.dma_start(
                out=wvu_sb, in_=wvu.rearrange("(c p) f -> p c f", p=128))

            cosq_sb = singles.tile([128, T], BF16)
            nc.sync.dma_start(out=cosq_sb, in_=cosq[:, :])
            sinq_sb = singles.tile([128, T], BF16)
            nc.sync.dma_start(out=sinq_sb, in_=sinq[:, :])
            tri_sb = singles.tile([128, 128], BF16)
            nc.sync.dma_start(out=tri_sb, in_=tri[:, :])
            sel_sb = singles.tile([66, 128], BF16)
            nc.sync.dma_start(out=sel_sb, in_=sel[:, :])
            lobo_sb = singles.tile([66, 1], F32)
            nc.sync.dma_start(out=lobo_sb, in_=lobo[:, :])
            c_sb = singles.tile([66, 1], F32)
            nc.scalar.activation(c_sb, lobo_sb, AF.Exp)
            wo_sb = singles.tile([128, 2, E], BF16)
            nc.gpsimd.dma_start(
                out=wo_sb, in_=wo.rearrange("(c p) e -> p c e", p=128))

            # ---------------- persistent activation tiles ----------------
            xt_sb = singles.tile([128, EC, T], BF16)
            xT_r = xT.rearrange("(c p) t -> p c t", p=128)
            latT_sb = singles.tile([128, CC, T], BF16)
            qT = [singles.tile([96, T], BF16, name=f"qT{h}") for h in range(HL)]
            kT = [singles.tile([96, T], BF16, name=f"kT{h}") for h in range(HL)]
            v_sb = singles.tile([128, NKC, HL, DH + 1], BF16)
            nc.vector.memset(v_sb[:, :, :, DH:DH + 1], 1.0)
            yT2 = singles.tile([128, 2, T], BF16)
            # head h row lives at partition 64*(h//2) + h%2
            dsum_sb = singles.tile([66, T], F32)
            emax_sb = singles.tile([66, T], F32)
            emst_sb = singles.tile([66, T], F32)
            nc.vector.memset(dsum_sb, 1.0)
            nc.vector.memset(emax_sb, 1.0)
            nc.vector.memset(emst_sb, 1.0)

            # =================== projections, per tg ===================
            for tg in range(NTG):
                ts = slice(tg * TG, (tg + 1) * TG)
                nc.sync.dma_start(
                    out=xt_sb[:, :, ts], in_=xT_r[:, :, ts])
                xts = [xt_sb[:, ec, ts] for ec in range(EC)]

                # --- latent (kv) halves + k_rope into one S tile ---
                skv = psp.tile([128, 3 * TG], F32, name="skv", tag="S", bufs=2)
                for ec in range(EC):
                    nc.tensor.matmul(
                        skv[:, 0:TG], wkvd_sb[:, ec, 0:128], xts[ec],
                        start=(ec == 0), stop=(ec == EC - 1))
                for ec in range(EC):
                    nc.tensor.matmul(
                        skv[:, TG:2 * TG], wkvd_sb[:, ec, 128:256], xts[ec],
                        start=(ec == 0), stop=(ec == EC - 1))
                for ec in range(EC):
                    nc.tensor.matmul(
                        skv[0:DR, 2 * TG:3 * TG], wkr_sb[:, ec, :], xts[ec],
                        start=(ec == 0), stop=(ec == EC - 1))
                nc.scalar.copy(
                    latT_sb[:, :, ts],
                    skv[:, 0:2 * TG].rearrange("p (c t) -> p c t", c=2))
                kr_pre = pool.tile([DR, TG], BF16, name="krp", tag="krp", bufs=2)
                nc.scalar.copy(kr_pre, skv[0:DR, 2 * TG:3 * TG])

                # k_rope rotate-half + tables
                kr_sw = pool.tile([DR, TG], BF16, name="krs", tag="krs", bufs=2)
                nc.sync.dma_start(out=kr_sw[0:16, :], in_=kr_pre[16:32, :])
                nc.sync.dma_start(out=kr_sw[16:32, :], in_=kr_pre[0:16, :])
                kr_m = pool.tile([DR, TG], BF16, name="krm", tag="krm", bufs=2)
                nc.vector.tensor_mul(kr_m, kr_pre, cosq_sb[0:DR, ts])
                nc.vector.tensor_mul(kr_sw, kr_sw, sinq_sb[0:DR, ts])
                nc.vector.tensor_add(kr_m, kr_m, kr_sw)
                for h in range(HL):
                    nc.sync.dma_start(out=kT[h][DH:96, ts], in_=kr_m)

                # --- q halves + q_rope into one S tile ---
                sq = psp.tile([128, 3 * TG], F32, name="sq", tag="S", bufs=2)
                for ec in range(EC):
                    nc.tensor.matmul(
                        sq[:, 0:TG], wq_sb[:, ec, 0:128], xts[ec],
                        start=(ec == 0), stop=(ec == EC - 1))
                for ec in range(EC):
                    nc.tensor.matmul(
                        sq[:, TG:2 * TG], wq_sb[:, ec, 128:256], xts[ec],
                        start=(ec == 0), stop=(ec == EC - 1))
                for ec in range(EC):
                    nc.tensor.matmul(
                        sq[:, 2 * TG:3 * TG], wqr_sb[:, ec, :], xts[ec],
                        start=(ec == 0), stop=(ec == EC - 1))
                stq = pool.tile([128, 2 * TG], BF16, name="stq", tag="stq", bufs=2)
                nc.scalar.copy(stq, sq[:, 0:2 * TG])
                nc.sync.dma_start(out=qT[0][0:DH, ts], in_=stq[0:DH, 0:TG])
                nc.sync.dma_start(out=qT[1][0:DH, ts], in_=stq[DH:128, 0:TG])
                nc.sync.dma_start(out=qT[2][0:DH, ts], in_=stq[0:DH, TG:2 * TG])
                nc.sync.dma_start(out=qT[3][0:DH, ts], in_=stq[DH:128, TG:2 * TG])
                rp_pre = pool.tile([128, TG], BF16, name="rpp", tag="rpp", bufs=2)
                nc.scalar.copy(rp_pre, sq[:, 2 * TG:3 * TG])
                rp_sw = pool.tile([128, TG], BF16, name="rps", tag="rps", bufs=2)
                for h in range(HL):
                    nc.sync.dma_start(
                        out=rp_sw[h * DR:h * DR + 16, :],
                        in_=rp_pre[h * DR + 16:h * DR + 32, :])
                    nc.sync.dma_start(
                        out=rp_sw[h * DR + 16:h * DR + 32, :],
                        in_=rp_pre[h * DR:h * DR + 16, :])
                rp_m = pool.tile([128, TG], BF16, name="rpm", tag="rpm", bufs=2)
                nc.vector.tensor_mul(rp_m, rp_pre, cosq_sb[:, ts])
                nc.vector.tensor_mul(rp_sw, rp_sw, sinq_sb[:, ts])
                nc.vector.tensor_add(rp_m, rp_m, rp_sw)
                for h in range(HL):
                    nc.sync.dma_start(
                        out=qT[h][DH:96, ts], in_=rp_m[h * DR:(h + 1) * DR, :])

                # --- k_c from latent ---
                skc = psp.tile([128, 3 * TG], F32, name="skc", tag="S", bufs=2)
                for cc in range(CC):
                    nc.tensor.matmul(
                        skc[:, 0:TG], wku_sb[:, cc, 0:128], latT_sb[:, cc, ts],
                        start=(cc == 0), stop=(cc == CC - 1))
                for cc in range(CC):
                    nc.tensor.matmul(
                        skc[:, TG:2 * TG], wku_sb[:, cc, 128:256],
                        latT_sb[:, cc, ts],
                        start=(cc == 0), stop=(cc == CC - 1))
                stk = pool.tile([128, 2 * TG], BF16, name="stk", tag="stq", bufs=2)
                nc.scalar.copy(stk, skc[:, 0:2 * TG])
                nc.sync.dma_start(out=kT[0][0:DH, ts], in_=stk[0:DH, 0:TG])
                nc.sync.dma_start(out=kT[1][0:DH, ts], in_=stk[DH:128, 0:TG])
                nc.sync.dma_start(out=kT[2][0:DH, ts], in_=stk[0:DH, TG:2 * TG])
                nc.sync.dma_start(out=kT[3][0:DH, ts], in_=stk[DH:128, TG:2 * TG])

                # --- V (natural layout) for this tg's 4 key chunks ---
                for half in range(2):
                    kc0 = 4 * tg + 2 * half
                    vps = psp.tile([128, TG], F32, name="vps", tag="Y", bufs=2)
                    for cc in range(CC):
                        nc.tensor.matmul(
                            vps[:, 0:256],
                            latT_sb[:, cc, kc0 * KC:(kc0 + 1) * KC],
                            wvu_sb[:, cc, :],
                            start=(cc == 0), stop=(cc == CC - 1))
                    for cc in range(CC):
                        nc.tensor.matmul(
                            vps[:, 256:512],
                            latT_sb[:, cc, (kc0 + 1) * KC:(kc0 + 2) * KC],
                            wvu_sb[:, cc, :],
                            start=(cc == 0), stop=(cc == CC - 1))
                    nc.scalar.copy(
                        v_sb[:, kc0:kc0 + 2, :, 0:DH],
                        vps.rearrange("p (k h d) -> p k h d", k=2, h=HL))

            # =================== attention ===================
            # two heads of a pair run as interleaved independent pipelines
            comb2 = singles.tile([128, 2, T], BF16)
            for hp2 in range(2):
                heads = (2 * hp2, 2 * hp2 + 1)
                for qg in range(NTG):
                    qs = slice(qg * TG, (qg + 1) * TG)
                    nkc = 4 * (qg + 1)
                    groups = [list(range(g0, min(g0 + 3, nkc)))
                              for g0 in range(0, nkc, 3)]
                    yp = {}
                    for h in heads:
                        yp[h] = psp.tile(
                            [128, TG], F32, name=f"yps{h % 2}", tag="Y",
                            bufs=2)
                    pts = {}
                    prev_pv = None

                    def joff(c):
                        # valid columns of chunk c start at j*KC (diag)
                        j = c - (nkc - 4)
                        return j * KC if j > 0 else 0

                    def emit_pv(gi):
                        for h in heads:
                            ptt, chunks = pts[(gi, h)]
                            for li, c in enumerate(chunks):
                                o = joff(c)
                                nc.tensor.matmul(
                                    yp[h][0:DH + 1, o:TG], v_sb[:, c, h, :],
                                    ptt[:, li, o:TG],
                                    start=(c == 0), stop=(c == nkc - 1),
                                    skip_group_check=True)

                    for gi, chunks in enumerate(groups):
                        sp = {}
                        for h in heads:
                            sp[h] = psp.tile(
                                [128, 3 * TG], F32, name=f"sps{h % 2}",
                                tag="S", bufs=2)
                            for li, c in enumerate(chunks):
                                o = joff(c)
                                nc.tensor.matmul(
                                    sp[h][:, li * TG + o:(li + 1) * TG],
                                    kT[h][:, c * KC:(c + 1) * KC],
                                    qT[h][:, qg * TG + o:(qg + 1) * TG])
                        if prev_pv is not None:
                            emit_pv(prev_pv)
                        for h in heads:
                            ptt = pool.tile(
                                [128, 3, TG], BF16, name=f"pt{h % 2}",
                                tag="pt", bufs=5)
                            pts[(gi, h)] = (ptt, chunks)
                            li = 0
                            while li < len(chunks):
                                c = chunks[li]
                                j = c - (nkc - 4)
                                if j < 0:
                                    l1 = li
                                    while (l1 < len(chunks)
                                           and chunks[l1] - (nkc - 4) < 0):
                                        l1 += 1
                                    nc.scalar.activation(
                                        ptt[:, li:l1, :],
                                        sp[h][:, li * TG:l1 * TG].rearrange(
                                            "p (k t) -> p k t", k=l1 - li),
                                        AF.Exp, scale=SCALE)
                                    li = l1
                                else:
                                    o = j * KC
                                    nc.scalar.activation(
                                        ptt[:, li, o:TG],
                                        sp[h][:, li * TG + o:(li + 1) * TG],
                                        AF.Exp, scale=SCALE)
                                    nc.gpsimd.tensor_mul(
                                        ptt[:, li, o:o + KC],
                                        ptt[:, li, o:o + KC], tri_sb)
                                    li += 1
                        for h in heads:
                            ptt, _ = pts[(gi, h)]
                            cslot = comb2[:, h % 2, :]
                            for li, c in enumerate(chunks):
                                o = joff(c)
                                if c == 0:
                                    nc.vector.tensor_copy(
                                        cslot[:, qs], ptt[:, li, :])
                                else:
                                    nc.vector.tensor_max(
                                        cslot[:, qg * TG + o:(qg + 1) * TG],
                                        cslot[:, qg * TG + o:(qg + 1) * TG],
                                        ptt[:, li, o:TG])
                        prev_pv = gi
                    emit_pv(prev_pv)

                    for h in heads:
                        # stage y + D, scatter to yT2 / dsum
                        hp = 64 * (h // 2) + h % 2
                        st65 = pool.tile(
                            [DH + 1, TG], F32, name="st65", tag="st65",
                            bufs=4)
                        nc.vector.tensor_copy(st65, yp[h][0:DH + 1, :])
                        nc.gpsimd.dma_start(
                            out=yT2[(h % 2) * DH:(h % 2 + 1) * DH,
                                    h // 2, qs],
                            in_=st65[0:DH, :])
                        nc.sync.dma_start(
                            out=dsum_sb[hp:hp + 1, qs],
                            in_=st65[DH:DH + 1, :])

                # per-head partition-max via 32x32 transpose + 2 reduces
                for h in heads:
                    hp = 64 * (h // 2) + h % 2
                    combT = pool.tile(
                        [128, T], BF16, name="combT", tag="combT", bufs=1)
                    nc.vector.transpose(combT, comb2[:, h % 2, :])
                    red = pool.tile(
                        [128, T // 32], F32, name="red", tag="red", bufs=2)
                    nc.vector.reduce_max(
                        red, combT.rearrange("p (b j) -> p b j", j=32),
                        axis=AX.X)
                    stkt = pool.tile(
                        [32, 4, T // 32], F32, name="stkt", tag="stkt",
                        bufs=2)
                    for a in range(4):
                        nc.sync.dma_start(
                            out=stkt[:, a, :],
                            in_=red[a * 32:(a + 1) * 32, :])
                    emf = pool.tile(
                        [32, T // 32], F32, name="emf", tag="emf", bufs=2)
                    nc.vector.reduce_max(
                        emf, stkt.rearrange("p a b -> p b a"), axis=AX.X)
                    nc.sync.dma_start(
                        out=emst_sb[hp:hp + 1, :].rearrange(
                            "p (i b) -> p i b", i=32),
                        in_=emf)

            # ========== denominators + normalize + out-proj, per tg ==========
            r_bf = singles.tile([66, T], BF16)
            emst_r = emst_sb.rearrange("p (i b) -> p i b", b=64)
            for tg in range(NTG):
                ts = slice(tg * TG, (tg + 1) * TG)
                # un-permute this chunk of the i-major per-block maxes
                nc.vector.tensor_copy(
                    emax_sb[:, ts].rearrange("p (b i) -> p i b", i=32),
                    emst_r[:, :, 16 * tg:16 * (tg + 1)])
                # d = dsum + C * emax (in place), r = exp(-ln d) on ACT
                nc.vector.scalar_tensor_tensor(
                    out=dsum_sb[:, ts], in0=emax_sb[:, ts], scalar=c_sb,
                    in1=dsum_sb[:, ts], op0=ALU.mult, op1=ALU.add)
                lnd = pool.tile([66, TG], F32, name="lnd", tag="lnd", bufs=2)
                nc.scalar.activation(lnd, dsum_sb[:, ts], AF.Ln)
                nc.scalar.activation(r_bf[:, ts], lnd, AF.Exp, scale=-1.0)
                for g in range(2):
                    bcps = psp.tile([128, TG], F32, name="bc", tag="Y", bufs=2)
                    nc.tensor.matmul(
                        bcps, sel_sb[64 * g:64 * g + 2, :],
                        r_bf[64 * g:64 * g + 2, ts])
                    nc.vector.tensor_mul(
                        yT2[:, g, ts], yT2[:, g, ts], bcps)
                for tt in range(4 * tg, 4 * (tg + 1)):
                    for eg in range(2):
                        ops = psp.tile(
                            [128, TG], F32, name="ops", tag="Y", bufs=2)
                        for fc in range(2):
                            nc.tensor.matmul(
                                ops, yT2[:, fc, tt * KC:(tt + 1) * KC],
                                wo_sb[:, fc, eg * TG:(eg + 1) * TG],
                                start=(fc == 0), stop=(fc == 1))
                        ost = pool.tile(
                            [128, TG], BF16, name="ost", tag="ost", bufs=3)
                        if (tt * 2 + eg) % 2 == 0:
                            nc.scalar.copy(ost, ops)
                        else:
                            nc.vector.tensor_copy(ost, ops)
                        nc.sync.dma_start(
                            out=out[tt * KC:(tt + 1) * KC,
                                    eg * TG:(eg + 1) * TG],
                            in_=ost)

    return nc


def _tri():
    x = np.arange(128)[:, None]
    y = np.arange(128)[None, :]
    return (x <= y).astype(np.float32)  # keep iff key row <= query col


def _sel():
    s = np.zeros((66, 128), dtype=np.float32)
    for base in (0, 64):
        s[base + 0, 0:64] = 1.0
        s[base + 1, 64:128] = 1.0
    return s


def _lobo66(lg):
    v = np.zeros((66, 1), dtype=np.float32)
    for h in range(HL):
        v[64 * (h // 2) + h % 2, 0] = lg[h]
    return v


def _rope_tables():
    half = DR // 2
    inv = 1.0 / (10000.0 ** (np.arange(half, dtype=np.float64) / half))
    ang = np.arange(T, dtype=np.float64)[:, None] * inv[None, :]  # (T, half)
    cos = np.cos(ang).T  # (half, T)
    sin = np.sin(ang).T
    cosk = np.concatenate([cos, cos], axis=0)                 # (32, T)
    sink = np.concatenate([-sin, sin], axis=0)
    cosq = np.tile(cosk, (HL, 1)).astype(np.float32)          # (128, T)
    sinq = np.tile(sink, (HL, 1)).astype(np.float32)
    return cosq, sinq


def kernel(x, Wq, Wqr, Wkr, Wkvd, Wku, Wvu, Wo, lobo_log):
    x = np.asarray(x, dtype=np.float32)
    Wq = np.asarray(Wq, dtype=np.float32)
    Wqr = np.asarray(Wqr, dtype=np.float32)
    Wkr = np.asarray(Wkr, dtype=np.float32)
    Wkvd = np.asarray(Wkvd, dtype=np.float32)
    Wku = np.asarray(Wku, dtype=np.float32)
    Wvu = np.asarray(Wvu, dtype=np.float32)
    Wo = np.asarray(Wo, dtype=np.float32)
    lobo_log = np.asarray(lobo_log, dtype=np.float32)

    if "nc" not in _CACHE:
        _CACHE["nc"] = _build_program()
    nc = _CACHE["nc"]

    cosq, sinq = _rope_tables()
    bf = lambda a: np.ascontiguousarray(a).astype(BF16NP)
    xTb = [bf(x[b].T) for b in range(B)]
    wkr_b, wkvd_b = bf(Wkr), bf(Wkvd)
    cosq_b, sinq_b = bf(cosq), bf(sinq)
    tri_b, sel_b = bf(_tri()), bf(_sel())
    in_maps = []
    for core in range(8):
        b, g = core // NG, core % NG
        hs = slice(g * HL * DH, (g + 1) * HL * DH)
        rs = slice(g * HL * DR, (g + 1) * HL * DR)
        in_maps.append({
            "xT": xTb[b],
            "wq": bf(Wq[:, hs]),
            "wqr": bf(Wqr[:, rs]),
            "wkr": wkr_b,
            "wkvd": wkvd_b,
            "wku": bf(Wku[:, hs]),
            "wvu": bf(Wvu[:, hs]),
            "wo": bf(Wo[hs, :]),
            "cosq": cosq_b, "sinq": sinq_b,
            "tri": tri_b, "sel": sel_b,
            "lobo": _lobo66(lobo_log[g * HL:(g + 1) * HL]),
        })

    trace = bool(os.environ.get("BASS_TRACE_KERNEL"))
    bkr = run_bass_kernel_spmd(
        nc, in_maps, core_ids=list(range(8)), trace=trace)
    if trace:
        print(f"HW exec time: {bkr.exec_time_ns} ns")
        if bkr.instructions_and_trace is not None:
            print("trace:", bkr.instructions_and_trace[1])
        _CACHE["last_result"] = bkr
    res = bkr.results
    out = np.zeros((B, T, E), dtype=np.float32)
    for core in range(8):
        out[core // NG] += np.asarray(res[core]["out"], dtype=np.float32)
    return out


# revision 19
# speedup vs baseline: 1.6464x; 1.0182x over previous
"""MLA (multi-head latent attention) Bass kernel for 8 trn2 NeuronCores.

Sharding: core = b*4 + g  (b in {0,1} batches, g in {0..3} head-groups of 4 heads).
Each core: projections from xT (bf16 matmuls), flash-style causal attention with
k-major scores (S^T) so exp'd probs feed PV directly, LOBO softmax
attn = exp(s) / (sum_k exp(s) + C*exp(max_k s)), row-parallel out-proj partial.
Host sums the 4 partials per batch.

v2 layout notes:
  - PSUM tags: S = [128,1536] (3 banks) x2, Y = [128,512] x2  -> 8 banks total.
  - Projections pack (kv0,kv1,kr) and (q01,q23,qr) each into one S tile.
  - Attention: per (h,qg) score chunks land 3-per-S-tile; one wide exp per
    off-diagonal run, granular exps + DVE-memset + gpsimd triangle-mask on the
    4 diagonal chunks.  Per-query running max kept in a [128,T] bf16 comb tile
    (DVE tensor_max); partition-max via ONE gpsimd tensor_reduce(axis=C).
  - Denominator D rides a ones-column in V through the PV matmul (row 64).
  - y normalization: r broadcast to 64-row blocks via a tiny f32r matmul with a
    0/1 selector lhsT; yT2 multiplied in place.
"""

import math
import os

import ml_dtypes
import numpy as np

BF16NP = ml_dtypes.bfloat16

import concourse.bass as bass
import concourse.bass_isa as bass_isa
import concourse.mybir as mybir
import concourse.tile as _tile_mod
from concourse.tile import TileContext
from concourse.vector_clock import ScopedClock, VectorClock
import bass_rust as _bass_rust
from concourse.bass_utils import run_bass_kernel_spmd

_N_PROCS = _bass_rust.N_PROCS


def _split_drain_and_barrier(self, tick_clock, wait_clock):
    """Replacement for TileContext._drain_and_barrier: the stock version puts
    the whole global vector clock (up to 27 sem waits) on one Drain, which this
    walrus rejects ("Too many sync wait commands").  Emit one Drain per
    outstanding processor instead."""
    gc = tick_clock.global_clock
    procs = [p for p in range(_N_PROCS) if gc[p] > 0]
    for p in procs:
        vc = VectorClock([gc[q] if q == p else 0 for q in range(_N_PROCS)])
        d = self.nc.sync.drain()
        wait_clock.add_sem_waits(d.ins, ScopedClock({None: vc}))
    self.nc.all_engine_barrier()
    popped = self.nc._tile_sem_poison_stack.pop()
    assert popped is self._sem_poison
    self.nc.clear_and_free_semaphores(list(self.sems.allocated().values()))
    self.nc.all_engine_barrier()


_tile_mod.TileContext._drain_and_barrier = _split_drain_and_barrier

# ---------------------------------------------------------------------------
# This walrus build enforces small per-instruction sync-wait budgets
# ("Too many sync wait commands").  Post-process the BIR JSON: any
# instruction carrying more than its budget of waits gets the excess
# hoisted onto same-engine Drain carriers inserted immediately before it
# (same program point on the engine's sequential stream -> semantics
# unchanged).
# ---------------------------------------------------------------------------
_orig_to_json_bytes = bass.Bass.to_json_bytes
_WAIT_LIMITS = {"Drain": 1, "DMACopy": 1}
_DEF_WAIT_LIMIT = 1


def _to_json_split_waits(self, *a, **kw):
    import json as _json
    data = _json.loads(_orig_to_json_bytes(self, *a, **kw))
    nid = 0
    for f in data.get("functions", []):
        for bb in f.get("blocks", []):
            out = []
            for inst in bb.get("instructions", []):
                si = inst.get("sync_info")
                if isinstance(si, dict):
                    w = si.get("on_wait")
                    if isinstance(w, list):
                        k = _WAIT_LIMITS.get(inst.get("opcode"), _DEF_WAIT_LIMIT)
                        if len(w) > k:
                            extra, keep = w[:-k], w[-k:]
                            for wt in extra:
                                out.append({
                                    "debug": inst.get("debug"),
                                    "engine": inst["engine"],
                                    "ins": [], "outs": [],
                                    "name": f"wsplit-{nid}",
                                    "opcode": "Drain",
                                    "sync_info": {"on_update": [],
                                                  "on_wait": [wt]},
                                })
                                nid += 1
                            si["on_wait"] = keep
                out.append(inst)
            bb["instructions"] = out
    return _json.dumps(data).encode()


bass.Bass.to_json_bytes = _to_json_split_waits

B, T, E = 2, 2048, 1024
H, DH = 16, 64
DKV = 256
DR = 32
HL = 4              # heads per core
NG = 4              # head groups
SCALE = 1.0 / math.sqrt(DH + DR)
TG = 512            # query-group width
KC = 128            # key-chunk width
NTG = T // TG       # 4
NKC = T // KC       # 16
EC = E // 128       # 8  e-chunks
CC = DKV // 128     # 2  latent chunks

F32 = mybir.dt.float32
F32R = mybir.dt.float32r
BF16 = mybir.dt.bfloat16
AF = mybir.ActivationFunctionType
ALU = mybir.AluOpType
AX = mybir.AxisListType

SWAP16 = list(range(16, 32)) + list(range(0, 16))

_CACHE = {}


def _r(ap):
    return ap.bitcast(F32R)


def _build_program():
    nc = bass.Bass()

    xT = nc.declare_dram_parameter("xT", [E, T], BF16, isOutput=False)
    wq = nc.declare_dram_parameter("wq", [E, HL * DH], BF16, isOutput=False)
    wqr = nc.declare_dram_parameter("wqr", [E, HL * DR], BF16, isOutput=False)
    wkr = nc.declare_dram_parameter("wkr", [E, DR], BF16, isOutput=False)
    wkvd = nc.declare_dram_parameter("wkvd", [E, DKV], BF16, isOutput=False)
    wku = nc.declare_dram_parameter("wku", [DKV, HL * DH], BF16, isOutput=False)
    wvu = nc.declare_dram_parameter("wvu", [DKV, HL * DH], BF16, isOutput=False)
    wo = nc.declare_dram_parameter("wo", [HL * DH, E], BF16, isOutput=False)
    cosq = nc.declare_dram_parameter("cosq", [HL * DR, T], BF16, isOutput=False)
    sinq = nc.declare_dram_parameter("sinq", [HL * DR, T], BF16, isOutput=False)
    tri = nc.declare_dram_parameter("tri", [128, 128], BF16, isOutput=False)
    sel = nc.declare_dram_parameter("sel", [66, 128], BF16, isOutput=False)
    lobo = nc.declare_dram_parameter("lobo", [66, 1], F32, isOutput=False)
    out = nc.declare_dram_parameter("out", [T, E], BF16, isOutput=True)

    with TileContext(nc) as tc:
        from contextlib import ExitStack

        with ExitStack() as ctx:
            singles = ctx.enter_context(tc.tile_pool(name="singles", bufs=1))
            pool = ctx.enter_context(tc.tile_pool(name="pool", bufs=2))
            psp = ctx.enter_context(tc.tile_pool(name="psp", bufs=1, space="PSUM"))

            # ---------------- weights (bf16 in SBUF) ----------------
            # tg0-critical loads lead each queue; wo is deferred to the end
            wq_sb = singles.tile([128, EC, HL * DH], BF16)
            nc.sync.dma_start(
                out=wq_sb, in_=wq.rearrange("(c p) f -> p c f", p=128))
            wkr_sb = singles.tile([128, EC, DR], BF16)
            nc.sync.dma_start(
                out=wkr_sb, in_=wkr.rearrange("(c p) f -> p c f", p=128))
            wkvd_sb = singles.tile([128, EC, DKV], BF16)
            nc.gpsimd.dma_start(
                out=wkvd_sb, in_=wkvd.rearrange("(c p) f -> p c f", p=128))
            wqr_sb = singles.tile([128, EC, HL * DR], BF16)
            nc.gpsimd.dma_start(
                out=wqr_sb, in_=wqr.rearrange("(c p) f -> p c f", p=128))
            wku_sb = singles.tile([128, CC, HL * DH], BF16)
            nc.gpsimd.dma_start(
                out=wku_sb, in_=wku.rearrange("(c p) f -> p c f", p=128))
            wvu_sb = singles.tile([128, CC, HL * DH], BF16)
            nc.gpsimd.dma_start(
                out=wvu_sb, in_=wvu.rearrange("(c p) f -> p c f", p=128))

            cosq_sb = singles.tile([128, T], BF16)
            nc.sync.dma_start(out=cosq_sb, in_=cosq[:, :])
            sinq_sb = singles.tile([128, T], BF16)
            nc.sync.dma_start(out=sinq_sb, in_=sinq[:, :])
            tri_sb = singles.tile([128, 128], BF16)
            nc.sync.dma_start(out=tri_sb, in_=tri[:, :])
            sel_sb = singles.tile([66, 128], BF16)
            nc.sync.dma_start(out=sel_sb, in_=sel[:, :])
            lobo_sb = singles.tile([66, 1], F32)
            nc.sync.dma_start(out=lobo_sb, in_=lobo[:, :])
            c_sb = singles.tile([66, 1], F32)
            nc.scalar.activation(c_sb, lobo_sb, AF.Exp)
            wo_sb = singles.tile([128, 2, E], BF16)
            nc.gpsimd.dma_start(
                out=wo_sb, in_=wo.rearrange("(c p) e -> p c e", p=128))

            # ---------------- persistent activation tiles ----------------
            xt_sb = singles.tile([128, EC, T], BF16)
            xT_r = xT.rearrange("(c p) t -> p c t", p=128)
            latT_sb = singles.tile([128, CC, T], BF16)
            # head h lives at slot SLOT[h] so staging DMAs can pair heads
            SLOT = [0, 2, 1, 3]
            qTall = singles.tile([96, HL, T], BF16)
            kTall = singles.tile([96, HL, T], BF16)
            v_sb = singles.tile([128, NKC, HL, DH + 1], BF16)
            nc.vector.memset(v_sb[:, :, :, DH:DH + 1], 1.0)
            yT2 = singles.tile([128, 2, T], BF16)
            # head h row lives at partition 64*(h//2) + h%2
            dsum_sb = singles.tile([66, T], F32)
            emax_sb = singles.tile([66, T], F32)
            emst_sb = singles.tile([66, T], F32)
            nc.vector.memset(dsum_sb, 1.0)
            nc.vector.memset(emax_sb, 1.0)
            nc.vector.memset(emst_sb, 1.0)

            # =================== projections, per tg ===================
            for tg in range(NTG):
                ts = slice(tg * TG, (tg + 1) * TG)
                nc.sync.dma_start(
                    out=xt_sb[:, :, ts], in_=xT_r[:, :, ts])
                xts = [xt_sb[:, ec, ts] for ec in range(EC)]

                # --- latent (kv) halves + k_rope into one S tile ---
                skv = psp.tile([128, 3 * TG], F32, name="skv", tag="S", bufs=2)
                for ec in range(EC):
                    nc.tensor.matmul(
                        skv[:, 0:TG], wkvd_sb[:, ec, 0:128], xts[ec],
                        start=(ec == 0), stop=(ec == EC - 1))
                for ec in range(EC):
                    nc.tensor.matmul(
                        skv[:, TG:2 * TG], wkvd_sb[:, ec, 128:256], xts[ec],
                        start=(ec == 0), stop=(ec == EC - 1))
                for ec in range(EC):
                    nc.tensor.matmul(
                        skv[0:DR, 2 * TG:3 * TG], wkr_sb[:, ec, :], xts[ec],
                        start=(ec == 0), stop=(ec == EC - 1))
                nc.scalar.copy(
                    latT_sb[:, :, ts],
                    skv[:, 0:2 * TG].rearrange("p (c t) -> p c t", c=2))
                kr_pre = pool.tile([DR, TG], BF16, name="krp", tag="krp", bufs=2)
                nc.scalar.copy(kr_pre, skv[0:DR, 2 * TG:3 * TG])

                # k_rope rotate-half + tables
                kr_sw = pool.tile([DR, TG], BF16, name="krs", tag="krs", bufs=2)
                nc.vector.stream_shuffle(
                    kr_sw, kr_pre, mask=SWAP16)
                kr_m = pool.tile([DR, TG], BF16, name="krm", tag="krm", bufs=2)
                nc.vector.tensor_mul(kr_m, kr_pre, cosq_sb[0:DR, ts])
                nc.vector.tensor_mul(kr_sw, kr_sw, sinq_sb[0:DR, ts])
                nc.vector.tensor_add(kr_m, kr_m, kr_sw)
                for h in range(HL):
                    nc.sync.dma_start(
                        out=kTall[DH:96, SLOT[h], ts], in_=kr_m)

                # --- q halves + q_rope into one S tile ---
                sq = psp.tile([128, 3 * TG], F32, name="sq", tag="S", bufs=2)
                for ec in range(EC):
                    nc.tensor.matmul(
                        sq[:, 0:TG], wq_sb[:, ec, 0:128], xts[ec],
                        start=(ec == 0), stop=(ec == EC - 1))
                for ec in range(EC):
                    nc.tensor.matmul(
                        sq[:, TG:2 * TG], wq_sb[:, ec, 128:256], xts[ec],
                        start=(ec == 0), stop=(ec == EC - 1))
                for ec in range(EC):
                    nc.tensor.matmul(
                        sq[:, 2 * TG:3 * TG], wqr_sb[:, ec, :], xts[ec],
                        start=(ec == 0), stop=(ec == EC - 1))
                stq = pool.tile([128, 2 * TG], BF16, name="stq", tag="stq", bufs=2)
                nc.scalar.copy(stq, sq[:, 0:2 * TG])
                nc.sync.dma_start(
                    out=qTall[0:DH, 0:2, ts],
                    in_=stq[0:DH, :].rearrange("p (k t) -> p k t", k=2))
                nc.sync.dma_start(
                    out=qTall[0:DH, 2:4, ts],
                    in_=stq[DH:128, :].rearrange("p (k t) -> p k t", k=2))
                rp_pre = pool.tile([128, TG], BF16, name="rpp", tag="rpp", bufs=2)
                nc.scalar.copy(rp_pre, sq[:, 2 * TG:3 * TG])
                rp_sw = pool.tile([128, TG], BF16, name="rps", tag="rps", bufs=2)
                nc.vector.stream_shuffle(rp_sw, rp_pre, mask=SWAP16)
                rp_m = pool.tile([128, TG], BF16, name="rpm", tag="rpm", bufs=2)
                nc.vector.tensor_mul(rp_m, rp_pre, cosq_sb[:, ts])
                nc.vector.tensor_mul(rp_sw, rp_sw, sinq_sb[:, ts])
                nc.vector.tensor_add(rp_m, rp_m, rp_sw)
                for h in range(HL):
                    nc.sync.dma_start(
                        out=qTall[DH:96, SLOT[h], ts],
                        in_=rp_m[h * DR:(h + 1) * DR, :])

                # --- k_c from latent ---
                skc = psp.tile([128, 3 * TG], F32, name="skc", tag="S", bufs=2)
                for cc in range(CC):
                    nc.tensor.matmul(
                        skc[:, 0:TG], wku_sb[:, cc, 0:128], latT_sb[:, cc, ts],
                        start=(cc == 0), stop=(cc == CC - 1))
                for cc in range(CC):
                    nc.tensor.matmul(
                        skc[:, TG:2 * TG], wku_sb[:, cc, 128:256],
                        latT_sb[:, cc, ts],
                        start=(cc == 0), stop=(cc == CC - 1))
                stk = pool.tile([128, 2 * TG], BF16, name="stk", tag="stq", bufs=2)
                nc.scalar.copy(stk, skc[:, 0:2 * TG])
                nc.sync.dma_start(
                    out=kTall[0:DH, 0:2, ts],
                    in_=stk[0:DH, :].rearrange("p (k t) -> p k t", k=2))
                nc.sync.dma_start(
                    out=kTall[0:DH, 2:4, ts],
                    in_=stk[DH:128, :].rearrange("p (k t) -> p k t", k=2))

                # --- V (natural layout) for this tg's 4 key chunks ---
                for half in range(2):
                    kc0 = 4 * tg + 2 * half
                    vps = psp.tile([128, TG], F32, name="vps", tag="Y", bufs=2)
                    for cc in range(CC):
                        nc.tensor.matmul(
                            vps[:, 0:256],
                            latT_sb[:, cc, kc0 * KC:(kc0 + 1) * KC],
                            wvu_sb[:, cc, :],
                            start=(cc == 0), stop=(cc == CC - 1))
                    for cc in range(CC):
                        nc.tensor.matmul(
                            vps[:, 256:512],
                            latT_sb[:, cc, (kc0 + 1) * KC:(kc0 + 2) * KC],
                            wvu_sb[:, cc, :],
                            start=(cc == 0), stop=(cc == CC - 1))
                    nc.scalar.copy(
                        v_sb[:, kc0:kc0 + 2, :, 0:DH],
                        vps.rearrange("p (k h d) -> p k h d", k=2, h=HL))

            # =================== attention ===================
            # two heads of a pair run as interleaved independent pipelines
            comb2 = singles.tile([128, 2, T], BF16)
            r_bf = singles.tile([66, T], BF16)
            emst_r = emst_sb.rearrange("p (i b) -> p i b", b=64)
            for hp2 in range(2):
                heads = (2 * hp2, 2 * hp2 + 1)
                for qg in range(NTG):
                    qs = slice(qg * TG, (qg + 1) * TG)
                    nkc = 4 * (qg + 1)
                    groups = [list(range(g0, min(g0 + 3, nkc)))
                              for g0 in range(0, nkc, 3)]
                    yp = {}
                    for h in heads:
                        yp[h] = psp.tile(
                            [128, TG], F32, name=f"yps{h % 2}", tag="Y",
                            bufs=2)
                    pts = {}
                    prev_pv = None

                    def joff(c):
                        # valid columns of chunk c start at j*KC (diag)
                        j = c - (nkc - 4)
                        return j * KC if j > 0 else 0

                    def emit_pv(gi):
                        for h in heads:
                            ptt, chunks = pts[(gi, h)]
                            for li, c in enumerate(chunks):
                                o = joff(c)
                                nc.tensor.matmul(
                                    yp[h][0:DH + 1, o:TG], v_sb[:, c, h, :],
                                    ptt[:, li, o:TG],
                                    start=(c == 0), stop=(c == nkc - 1),
                                    skip_group_check=True)

                    for gi, chunks in enumerate(groups):
                        sp = {}
                        for h in heads:
                            sp[h] = psp.tile(
                                [128, 3 * TG], F32, name=f"sps{h % 2}",
                                tag="S", bufs=2)
                            for li, c in enumerate(chunks):
                                o = joff(c)
                                nc.tensor.matmul(
                                    sp[h][:, li * TG + o:(li + 1) * TG],
                                    kTall[:, SLOT[h], c * KC:(c + 1) * KC],
                                    qTall[:, SLOT[h],
                                          qg * TG + o:(qg + 1) * TG])
                        if prev_pv is not None:
                            emit_pv(prev_pv)
                        for h in heads:
                            ptt = pool.tile(
                                [128, 3, TG], BF16, name=f"pt{h % 2}",
                                tag="pt", bufs=5)
                            pts[(gi, h)] = (ptt, chunks)
                            li = 0
                            while li < len(chunks):
                                c = chunks[li]
                                j = c - (nkc - 4)
                                if j < 0:
                                    l1 = li
                                    while (l1 < len(chunks)
                                           and chunks[l1] - (nkc - 4) < 0):
                                        l1 += 1
                                    nc.scalar.activation(
                                        ptt[:, li:l1, :],
                                        sp[h][:, li * TG:l1 * TG].rearrange(
                                            "p (k t) -> p k t", k=l1 - li),
                                        AF.Exp, scale=SCALE)
                                    li = l1
                                else:
                                    o = j * KC
                                    nc.scalar.activation(
                                        ptt[:, li, o:TG],
                                        sp[h][:, li * TG + o:(li + 1) * TG],
                                        AF.Exp, scale=SCALE)
                                    nc.gpsimd.tensor_mul(
                                        ptt[:, li, o:o + KC],
                                        ptt[:, li, o:o + KC], tri_sb)
                                    li += 1
                        for h in heads:
                            ptt, _ = pts[(gi, h)]
                            cslot = comb2[:, h % 2, :]
                            for li, c in enumerate(chunks):
                                o = joff(c)
                                if c == 0:
                                    nc.vector.tensor_copy(
                                        cslot[:, qs], ptt[:, li, :])
                                else:
                                    nc.vector.tensor_max(
                                        cslot[:, qg * TG + o:(qg + 1) * TG],
                                        cslot[:, qg * TG + o:(qg + 1) * TG],
                                        ptt[:, li, o:TG])
                        prev_pv = gi
                    emit_pv(prev_pv)

                    for h in heads:
                        # stage y + D, scatter to yT2 / dsum
                        hp = 64 * (h // 2) + h % 2
                        st65 = pool.tile(
                            [DH + 1, TG], F32, name="st65", tag="st65",
                            bufs=4)
                        nc.vector.tensor_copy(st65, yp[h][0:DH + 1, :])
                        nc.gpsimd.dma_start(
                            out=yT2[(h % 2) * DH:(h % 2 + 1) * DH,
                                    h // 2, qs],
                            in_=st65[0:DH, :])
                        nc.sync.dma_start(
                            out=dsum_sb[hp:hp + 1, qs],
                            in_=st65[DH:DH + 1, :])

                # per-head partition-max via 32x32 transpose + 2 reduces
                for h in heads:
                    hp = 64 * (h // 2) + h % 2
                    combT = pool.tile(
                        [128, T], BF16, name="combT", tag="combT", bufs=1)
                    nc.vector.transpose(combT, comb2[:, h % 2, :])
                    red = pool.tile(
                        [128, T // 32], F32, name="red", tag="red", bufs=2)
                    nc.vector.reduce_max(
                        red, combT.rearrange("p (b j) -> p b j", j=32),
                        axis=AX.X)
                    stkt = pool.tile(
                        [32, 4, T // 32], F32, name="stkt", tag="stkt",
                        bufs=2)
                    for a in range(4):
                        nc.sync.dma_start(
                            out=stkt[:, a, :],
                            in_=red[a * 32:(a + 1) * 32, :])
                    emf = pool.tile(
                        [32, T // 32], F32, name="emf", tag="emf", bufs=2)
                    nc.vector.reduce_max(
                        emf, stkt.rearrange("p a b -> p b a"), axis=AX.X)
                    nc.sync.dma_start(
                        out=emst_sb[hp:hp + 1, :].rearrange(
                            "p (i b) -> p i b", i=32),
                        in_=emf)

                # this pair's denominator chain (rows 64*hp2 .. +2) can run
                # during the next pair's attention / the out projection
                rows = slice(64 * hp2, 64 * hp2 + 2)
                for tg in range(NTG):
                    ts = slice(tg * TG, (tg + 1) * TG)
                    nc.vector.tensor_copy(
                        emax_sb[rows, ts].rearrange("p (b i) -> p i b", i=32),
                        emst_r[rows, :, 16 * tg:16 * (tg + 1)])
                    nc.vector.scalar_tensor_tensor(
                        out=dsum_sb[rows, ts], in0=emax_sb[rows, ts],
                        scalar=c_sb[rows, :], in1=dsum_sb[rows, ts],
                        op0=ALU.mult, op1=ALU.add)
                    lnd = pool.tile(
                        [2, TG], F32, name="lnd", tag="lnd", bufs=2)
                    nc.scalar.activation(lnd, dsum_sb[rows, ts], AF.Ln)
                    nc.scalar.activation(
                        r_bf[rows, ts], lnd, AF.Exp, scale=-1.0)

            # ========== normalize + out-proj, per tg ==========
            for tg in range(NTG):
                ts = slice(tg * TG, (tg + 1) * TG)
                for g in range(2):
                    bcps = psp.tile([128, TG], F32, name="bc", tag="Y", bufs=2)
                    nc.tensor.matmul(
                        bcps, sel_sb[64 * g:64 * g + 2, :],
                        r_bf[64 * g:64 * g + 2, ts])
                    nc.vector.tensor_mul(
                        yT2[:, g, ts], yT2[:, g, ts], bcps)
                for tt in range(4 * tg, 4 * (tg + 1)):
                    for eg in range(2):
                        ops = psp.tile(
                            [128, TG], F32, name="ops", tag="Y", bufs=2)
                        for fc in range(2):
                            nc.tensor.matmul(
                                ops, yT2[:, fc, tt * KC:(tt + 1) * KC],
                                wo_sb[:, fc, eg * TG:(eg + 1) * TG],
                                start=(fc == 0), stop=(fc == 1))
                        ost = pool.tile(
                            [128, TG], BF16, name="ost", tag="ost", bufs=3)
                        if (tt * 2 + eg) % 2 == 0:
                            nc.scalar.copy(ost, ops)
                        else:
                            nc.vector.tensor_copy(ost, ops)
                        nc.sync.dma_start(
                            out=out[tt * KC:(tt + 1) * KC,
                                    eg * TG:(eg + 1) * TG],
                            in_=ost)

    return nc


def _tri():
    x = np.arange(128)[:, None]
    y = np.arange(128)[None, :]
    return (x <= y).astype(np.float32)  # keep iff key row <= query col


def _sel():
    s = np.zeros((66, 128), dtype=np.float32)
    for base in (0, 64):
        s[base + 0, 0:64] = 1.0
        s[base + 1, 64:128] = 1.0
    return s


def _lobo66(lg):
    v = np.zeros((66, 1), dtype=np.float32)
    for h in range(HL):
        v[64 * (h // 2) + h % 2, 0] = lg[h]
    return v


def _rope_tables():
    half = DR // 2
    inv = 1.0 / (10000.0 ** (np.arange(half, dtype=np.float64) / half))
    ang = np.arange(T, dtype=np.float64)[:, None] * inv[None, :]  # (T, half)
    cos = np.cos(ang).T  # (half, T)
    sin = np.sin(ang).T
    cosk = np.concatenate([cos, cos], axis=0)                 # (32, T)
    sink = np.concatenate([-sin, sin], axis=0)
    cosq = np.tile(cosk, (HL, 1)).astype(np.float32)          # (128, T)
    sinq = np.tile(sink, (HL, 1)).astype(np.float32)
    return cosq, sinq


def kernel(x, Wq, Wqr, Wkr, Wkvd, Wku, Wvu, Wo, lobo_log):
    x = np.asarray(x, dtype=np.float32)
    Wq = np.asarray(Wq, dtype=np.float32)
    Wqr = np.asarray(Wqr, dtype=np.float32)
    Wkr = np.asarray(Wkr, dtype=np.float32)
    Wkvd = np.asarray(Wkvd, dtype=np.float32)
    Wku = np.asarray(Wku, dtype=np.float32)
    Wvu = np.asarray(Wvu, dtype=np.float32)
    Wo = np.asarray(Wo, dtype=np.float32)
    lobo_log = np.asarray(lobo_log, dtype=np.float32)

    if "nc" not in _CACHE:
        _CACHE["nc"] = _build_program()
    nc = _CACHE["nc"]

    cosq, sinq = _rope_tables()
    bf = lambda a: np.ascontiguousarray(a).astype(BF16NP)
    xTb = [bf(x[b].T) for b in range(B)]
    wkr_b, wkvd_b = bf(Wkr), bf(Wkvd)
    cosq_b, sinq_b = bf(cosq), bf(sinq)
    tri_b, sel_b = bf(_tri()), bf(_sel())
    in_maps = []
    for core in range(8):
        b, g = core // NG, core % NG
        hs = slice(g * HL * DH, (g + 1) * HL * DH)
        rs = slice(g * HL * DR, (g + 1) * HL * DR)
        in_maps.append({
            "xT": xTb[b],
            "wq": bf(Wq[:, hs]),
            "wqr": bf(Wqr[:, rs]),
            "wkr": wkr_b,
            "wkvd": wkvd_b,
            "wku": bf(Wku[:, hs]),
            "wvu": bf(Wvu[:, hs]),
            "wo": bf(Wo[hs, :]),
            "cosq": cosq_b, "sinq": sinq_b,
            "tri": tri_b, "sel": sel_b,
            "lobo": _lobo66(lobo_log[g * HL:(g + 1) * HL]),
        })

    trace = bool(os.environ.get("BASS_TRACE_KERNEL"))
    bkr = run_bass_kernel_spmd(
        nc, in_maps, core_ids=list(range(8)), trace=trace)
    if trace:
        print(f"HW exec time: {bkr.exec_time_ns} ns")
        if bkr.instructions_and_trace is not None:
            print("trace:", bkr.instructions_and_trace[1])
        _CACHE["last_result"] = bkr
    res = bkr.results
    out = np.zeros((B, T, E), dtype=np.float32)
    for core in range(8):
        out[core // NG] += np.asarray(res[core]["out"], dtype=np.float32)
    return out
